# revision 1
# baseline (speedup 1.0000x reference)
"""GAT (2-block, 3-layer) Trainium2 Bass kernel, 8-core SPMD.

Sharding: target-node rows (n) split across 8 cores (256 rows each).
Per layer, each core computes h = x @ W for ALL source nodes (needs the
full activation, obtained via AllGather), then row-local masked softmax
attention + aggregation for its 256 target rows.  The aggregation matmul
produces the TRANSPOSED activation [hc, n_own] which is exactly the
layout needed as lhsT for the next layer -- no transposes anywhere.
Final pooled vectors are partial-summed per core and reduced on host.

Self-contained: hardcodes all shapes; only needs /opt/trn_rl_repo.
"""
import sys
from contextlib import ExitStack

import numpy as np

sys.path.insert(0, "/opt/trn_rl_repo")

import concourse.bass as bass  # noqa: E402
import concourse.bacc as bacc  # noqa: E402
import concourse.tile as tile  # noqa: E402
from concourse import mybir  # noqa: E402
from concourse.bass_utils import run_bass_kernel_spmd  # noqa: E402

N = 2048
FIN = 128
H = 8
NCORES = 8
R = N // NCORES          # 256 target rows per core
MT = N // 128            # 16 source m-tiles
FP32 = mybir.dt.float32
FP16 = mybir.dt.float16

# (name, fan_in, C) ; blocks: 0 = layers *1x (C=64), 1 = *2x (C=32)
LAYERS = [("11", 128, 64), ("12", 512, 64), ("13", 512, 64),
          ("21", 128, 32), ("22", 256, 32), ("23", 256, 32)]
# emission order interleaves the two independent blocks so one block's
# compute hides the other's AllGather transition
SCHED = [("11", 128, 64, None), ("21", 128, 32, None),
         ("12", 512, 64, "11"), ("22", 256, 32, "21"),
         ("13", 512, 64, "12"), ("23", 256, 32, "22")]

_NC_CACHE = {}
DEBUG = False
# pointwise variant per (m_tile % len): A=ACT prelu+exp; B=DVE lrelu + ACT
# exp-half + DVE square-mask; G2=DVE s02 + GP max + ACT exp + GP mask;
# GM=A but mask on GP.  hcopy engine: a=ACT, v=DVE.
VARIANTS = ["A", "B", "B", "A", "B", "B", "A", "B"]
HCOPY = {"A": "a", "B": "a", "G2": "a", "GM": "a", "GB": "a"}


def _build(repeat=1, no_collective=False):
    nc = bacc.Bacc("TRN2", target_bir_lowering=False, debug=False,
                   num_devices=NCORES)

    # ---------------- DRAM I/O ----------------
    xT0_d = nc.dram_tensor("xT0", [FIN, N], FP16, kind="ExternalInput")
    xo0_d = nc.dram_tensor("xo0", [FIN, R], FP16, kind="ExternalInput")
    aT_d = nc.dram_tensor("aT", [N, R], FP16, kind="ExternalInput")
    es1_d = nc.dram_tensor("es1", [N], FP16, kind="ExternalInput")
    es2_d = nc.dram_tensor("es2", [N], FP16, kind="ExternalInput")
    wc_d, ws_d, b_d = {}, {}, {}
    for (nm, F, C) in LAYERS:
        HC = H * C
        # Wcat = [W.reshape(F, HC) | Wt]  (Wt = einsum(W, at))
        wc_d[nm] = nc.dram_tensor(f"Wc{nm}", [F, HC + H], FP16,
                                  kind="ExternalInput")
        ws_d[nm] = nc.dram_tensor(f"Ws{nm}", [F, H], FP16, kind="ExternalInput")
        b_d[nm] = nc.dram_tensor(f"b{nm}", [HC], FP32, kind="ExternalInput")
    pool_d = nc.dram_tensor("pool", [768], FP32, kind="ExternalOutput")

    dbg_d = {}
    if DEBUG:
        for li, (nm, F, C) in enumerate(LAYERS):
            OC = (H * C) // 128
            dbg_d[nm] = nc.dram_tensor(f"dbg{nm}", [128, OC * R], FP16,
                                       kind="ExternalOutput")

    # internal DRAM: es scratch per layer + allgather buffers per transition
    es_scr, ag_in, ag_out = {}, {}, {}
    for rep in range(repeat):
        for li, (nm, F, C) in enumerate(LAYERS):
            key = (rep, nm)
            es_scr[key] = nc.dram_tensor(f"esscr{rep}_{nm}", [N], FP16,
                                         kind="Internal")
            if li % 3 != 2:
                HC = H * C
                ag_in[key] = nc.dram_tensor(f"agin{rep}_{nm}", [HC, R], FP16,
                                            kind="Internal")
                ag_out[key] = nc.dram_tensor(
                    f"agout{rep}_{nm}", [NCORES * HC, R], FP16,
                    kind="Internal", addr_space="Shared")

    with tile.TileContext(nc) as tc:
        with ExitStack() as ctx:
            pl = lambda **kw: ctx.enter_context(tc.tile_pool(**kw))  # noqa: E731
            constp = pl(name="const", bufs=1)
            wmp = pl(name="wm", bufs=2)
            wsmp = pl(name="wsm", bufs=2)
            xtb1p = pl(name="xtb1", bufs=1)
            xtb2p = pl(name="xtb2", bufs=1)
            hp = pl(name="hsb", bufs=4)
            esbp = pl(name="esb", bufs=2)
            essbp = pl(name="essb", bufs=2)
            etp = pl(name="etsb", bufs=4)
            sp = pl(name="s", bufs=2)
            up = pl(name="u", bufs=3)
            Pp = pl(name="P", bufs=2)
            pp = pl(name="p", bufs=4)
            xnp = pl(name="xn", bufs=2)
            dvp = pl(name="dv", bufs=4)
            dinvp = pl(name="dinv", bufs=2)
            dinvbp = pl(name="dinvb", bufs=2)
            poutp = pl(name="pout", bufs=1)
            php = pl(name="ph", bufs=2, space="PSUM")
            pep = pl(name="pe", bufs=1, space="PSUM")
            paggp = pl(name="pagg", bufs=4, space="PSUM")
            pDp = pl(name="pD", bufs=1, space="PSUM")

            # -------- prologue: resident inputs --------
            aT_sb = constp.tile([128, MT * R], FP16)          # [m | (mt, n)]
            aT_v = aT_d[:].rearrange("(t p) n -> p t n", p=128)
            _eng = [nc.gpsimd, nc.scalar, nc.gpsimd, nc.scalar]
            for j in range(4):
                _eng[j].dma_start(
                    aT_sb[:].rearrange("p (t n) -> p t n", t=MT)
                    [:, 4 * j:4 * j + 4, :],
                    aT_v[:, 4 * j:4 * j + 4, :])
            xT0_sb = constp.tile([128, N], FP16)
            nc.sync.dma_start(xT0_sb[:], xT0_d[:])
            xo0_sb = constp.tile([128, R], FP16)
            nc.sync.dma_start(xo0_sb[:], xo0_d[:])
            ones_sb = constp.tile([128, 1], FP16)
            nc.gpsimd.memset(ones_sb[:], 1.0)

            for rep in range(repeat):
                xn_prev = [None, None]   # per block: own transposed activation
                for li, (nm, F, C, prev) in enumerate(SCHED):
                    HC = H * C
                    FC = F // 128        # input chunks (of fan-in)
                    OC = HC // 128       # output chunks (of hc rows)
                    W2 = HC + H          # wcat width
                    blk = 0 if nm[0] == "1" else 1
                    lyr = int(nm[1]) - 1
                    fuse_et = (HC + H) <= 512   # block2: et inside h-matmul

                    # -------- weights --------
                    wc_sb = wmp.tile([128, FC * W2], FP16, tag="wm")
                    nc.gpsimd.dma_start(
                        wc_sb[:].rearrange("p (c d) -> p c d", c=FC),
                        wc_d[nm][:].rearrange("(c p) d -> p c d", p=128))
                    ws_sb = wsmp.tile([128, FC * H], FP16, tag="ws")
                    nc.gpsimd.dma_start(
                        ws_sb[:].rearrange("p (c d) -> p c d", c=FC),
                        ws_d[nm][:].rearrange("(c p) d -> p c d", p=128))
                    b_sb = wsmp.tile([128, OC], FP32, tag="b")
                    nc.gpsimd.dma_start(
                        b_sb[:], b_d[nm][:].rearrange("(c p) -> p c", p=128))

                    # -------- xT (all nodes, transposed) --------
                    if lyr == 0:
                        xT_sb = xT0_sb
                        xo_ap = xo0_sb
                    else:
                        pool_x = xtb1p if blk == 0 else xtb2p
                        xT_sb = pool_x.tile([128, FC * N], FP16, tag=f"xt{blk}")
                        gsrc = ag_out[(rep, prev)]
                        gv = gsrc[:].rearrange("(r c p) n -> p c r n",
                                               r=NCORES, p=128)
                        for fc in range(FC):
                            nc.sync.dma_start(
                                xT_sb[:, fc * N:(fc + 1) * N]
                                .rearrange("p (r n) -> p r n", r=NCORES),
                                gv[:, fc, :, :])
                        xo_ap = xn_prev[blk]

                    # -------- es chain --------
                    if lyr == 0:
                        es_src = es1_d if blk == 0 else es2_d
                    else:
                        es_src = es_scr[(rep, nm)]
                        pes = pep.tile([8, R], FP32, tag="pe")
                        for fc in range(FC):
                            nc.tensor.matmul(
                                pes[:], ws_sb[:, fc * H:(fc + 1) * H],
                                xo_ap[:, fc * R:(fc + 1) * R],
                                start=(fc == 0), stop=(fc == FC - 1))
                        es_sb = essbp.tile([8, R], FP16, tag="es")
                        nc.scalar.copy(es_sb[:], pes[:])
                        nc.gpsimd.dma_start(
                            es_src[:].rearrange("(h n) -> h n", h=8), es_sb[:])
                    esb = esbp.tile([128, N], FP16, tag="esb")
                    for j in range(2):
                        nc.sync.dma_start(
                            esb[:, j * 1024:(j + 1) * 1024],
                            es_src[j * 1024:(j + 1) * 1024][None, :]
                            .to_broadcast((128, 1024)))

                    # -------- aggregation psum (live across m loop) --------
                    agg_q = []
                    n_agg = 4 if C == 64 else 2
                    for _qi in range(n_agg):
                        agg_t = paggp.tile([128, 512], FP32, tag="agg")
                        agg_q.append(agg_t)
                    pD = pDp.tile([128, 512], FP32, tag="pD")

                    for i in range(MT):
                        # h (+ et fused for block2) for m-tile i
                        ph = php.tile([128, W2 if fuse_et else HC], FP32,
                                      tag="ph")
                        rw = W2 if fuse_et else HC
                        for fc in range(FC):
                            lhs = xT_sb[:, fc * N + i * 128:
                                        fc * N + (i + 1) * 128]
                            nc.tensor.matmul(
                                ph[:], lhs, wc_sb[:, fc * W2: fc * W2 + rw],
                                start=(fc == 0), stop=(fc == FC - 1))
                        if not fuse_et:
                            pet = pep.tile([128, H], FP32, tag="pe")
                            for fc in range(FC):
                                lhs = xT_sb[:, fc * N + i * 128:
                                            fc * N + (i + 1) * 128]
                                nc.tensor.matmul(
                                    pet[:], lhs,
                                    wc_sb[:, fc * W2 + HC: (fc + 1) * W2],
                                    start=(fc == 0), stop=(fc == FC - 1))
                        var = VARIANTS[i % len(VARIANTS)]
                        h_sb = hp.tile([128, HC], FP16, tag="h")
                        et_t = etp.tile([128, H], FP32, tag="et")
                        nc.scalar.copy(h_sb[:], ph[:, 0:HC])
                        if fuse_et:
                            nc.scalar.copy(et_t[:], ph[:, HC:W2])
                        else:
                            nc.scalar.copy(et_t[:], pet[:])
                        et_ap = et_t

                        # pointwise: p = aT * exp(lrelu_0.2(es + et))
                        # variant A (ACT-heavy): Prelu + Exp on ACT, mask DVE
                        # variant B (DVE-heavy): lrelu = max(s, .2s) on DVE,
                        #   exp(u/2) on ACT, then p = (A2*aT)*A2 on DVE
                        s_t = sp.tile([128, N], FP16, tag="s")
                        for h in range(H):
                            nc.vector.tensor_scalar(
                                s_t[:, h * R:(h + 1) * R],
                                esb[:, h * R:(h + 1) * R],
                                et_ap[:, h: h + 1], None,
                                mybir.AluOpType.add)
                        aT_ap = (aT_sb[:, i * R:(i + 1) * R][:, None, :]
                                 .to_broadcast((128, H, R)))
                        p_t = pp.tile([128, N], FP16, tag="p")
                        if var in ("A", "GM"):
                            u_t = up.tile([128, N], FP16, tag="u")
                            nc.scalar.activation(
                                u_t[:], s_t[:],
                                mybir.ActivationFunctionType.Prelu, alpha=0.2)
                            P_t = Pp.tile([128, N], FP16, tag="P")
                            nc.scalar.activation(
                                P_t[:], u_t[:],
                                mybir.ActivationFunctionType.Exp)
                            eng = nc.vector if var == "A" else nc.gpsimd
                            eng.tensor_tensor(
                                p_t[:].rearrange("p (h n) -> p h n", h=H),
                                P_t[:].rearrange("p (h n) -> p h n", h=H),
                                aT_ap, mybir.AluOpType.mult)
                        elif var == "GB":
                            # DVE lrelu, ACT exp, GP mask
                            s2_t = up.tile([128, N], FP16, tag="u")
                            nc.vector.tensor_scalar(
                                s2_t[:], s_t[:], 0.2, None,
                                mybir.AluOpType.mult)
                            u2_t = up.tile([128, N], FP16, tag="u")
                            nc.vector.tensor_tensor(
                                u2_t[:], s_t[:], s2_t[:],
                                mybir.AluOpType.max)
                            P_t = Pp.tile([128, N], FP16, tag="P")
                            nc.scalar.activation(
                                P_t[:], u2_t[:],
                                mybir.ActivationFunctionType.Exp)
                            nc.gpsimd.tensor_tensor(
                                p_t[:].rearrange("p (h n) -> p h n", h=H),
                                P_t[:].rearrange("p (h n) -> p h n", h=H),
                                aT_ap, mybir.AluOpType.mult)
                        elif var == "B":
                            # lrelu on DVE (max(s, .2s)), exp on ACT, mask DVE
                            s2_t = up.tile([128, N], FP16, tag="u")
                            nc.vector.tensor_scalar(
                                s2_t[:], s_t[:], 0.2, None,
                                mybir.AluOpType.mult)
                            u2_t = up.tile([128, N], FP16, tag="u")
                            nc.vector.tensor_tensor(
                                u2_t[:], s_t[:], s2_t[:],
                                mybir.AluOpType.max)
                            P_t = Pp.tile([128, N], FP16, tag="P")
                            nc.scalar.activation(
                                P_t[:], u2_t[:],
                                mybir.ActivationFunctionType.Exp)
                            nc.vector.tensor_tensor(
                                p_t[:].rearrange("p (h n) -> p h n", h=H),
                                P_t[:].rearrange("p (h n) -> p h n", h=H),
                                aT_ap, mybir.AluOpType.mult)
                        else:  # G2: DVE s02, GP max, ACT exp, GP mask
                            s2_t = up.tile([128, N], FP16, tag="u")
                            nc.vector.tensor_scalar(
                                s2_t[:], s_t[:], 0.2, None,
                                mybir.AluOpType.mult)
                            nc.gpsimd.tensor_tensor(
                                s_t[:], s_t[:], s2_t[:],
                                mybir.AluOpType.max)
                            P_t = Pp.tile([128, N], FP16, tag="P")
                            nc.scalar.activation(
                                P_t[:], s_t[:],
                                mybir.ActivationFunctionType.Exp)
                            nc.gpsimd.tensor_tensor(
                                p_t[:].rearrange("p (h n) -> p h n", h=H),
                                P_t[:].rearrange("p (h n) -> p h n", h=H),
                                aT_ap, mybir.AluOpType.mult)

                        # aggregation: 2 heads per matmul ([128, 512] rhs).
                        # One accumulation group per (partition-range, bank).
                        for j in range(4):
                            lhsT = h_sb[:, j * 2 * C:(j + 1) * 2 * C]
                            rhs = p_t[:, j * 512:(j + 1) * 512]
                            if C == 64:
                                nc.tensor.matmul(
                                    agg_q[j][:, :], lhsT, rhs,
                                    start=(i == 0), stop=(i == MT - 1),
                                    tile_position=(0, 0))
                            else:
                                pb = (j % 2) * 64
                                nc.tensor.matmul(
                                    agg_q[j // 2][pb:pb + 64, :], lhsT, rhs,
                                    start=(i == 0), stop=(i == MT - 1),
                                    tile_position=(0, pb),
                                    skip_group_check=(pb > 0))
                        for j in range(4):
                            nc.tensor.matmul(
                                pD[32 * j:32 * j + 1, :],
                                ones_sb[:],
                                p_t[:, j * 512:(j + 1) * 512],
                                start=(i == 0), stop=(i == MT - 1),
                                tile_position=(0, 32 * j),
                                skip_group_check=(j > 0))

                    # -------- finalize: alpha-normalize + bias + relu ------
                    dinv = dinvp.tile([1, N], FP32, tag="dinv")
                    for j in range(4):
                        nc.vector.reciprocal(dinv[0:1, j * 512:(j + 1) * 512],
                                             pD[32 * j:32 * j + 1, :])
                    dinvb = dinvbp.tile([128, N], FP32, tag="dinvb")
                    nc.gpsimd.partition_broadcast(dinvb[:], dinv[0:1, :])
                    xn = xnp.tile([128, OC * R], FP16, tag=f"xn{blk}")
                    hpc = 128 // C  # heads per 128-row chunk
                    for t in range(OC):
                        for k in range(hpc):
                            pb, h = k * C, t * hpc + k
                            fo = (k % 2) * 256
                            # b1: chunk t = pair tile t; b2: tile t, see map
                            src = agg_q[t][pb:pb + C, fo:fo + R]
                            dv = dvp.tile([128, R], FP32, tag="dv")
                            nc.vector.tensor_tensor(
                                dv[pb:pb + C, :], src,
                                dinvb[pb:pb + C, h * R:(h + 1) * R],
                                mybir.AluOpType.mult)
                            nc.scalar.activation(
                                xn[pb:pb + C, t * R:(t + 1) * R],
                                dv[pb:pb + C, :],
                                mybir.ActivationFunctionType.Relu,
                                bias=b_sb[pb:pb + C, t:t + 1])

                    if DEBUG:
                        nc.sync.dma_start(dbg_d[nm][:, 0:OC * R], xn[:])
                    if lyr == 2:
                        # global pool: partial sum over own 256 rows
                        po = poutp.tile([128, OC], FP32, tag=f"po{blk}")
                        for t in range(OC):
                            nc.vector.tensor_reduce(
                                po[:, t:t + 1], xn[:, t * R:(t + 1) * R],
                                axis=mybir.AxisListType.X,
                                op=mybir.AluOpType.add)
                        off = 0 if blk == 0 else 512
                        nc.sync.dma_start(
                            pool_d[off:off + HC].rearrange("(c p) -> p c",
                                                           p=128),
                            po[:])
                    else:
                        xn_prev[blk] = xn
                        nc.gpsimd.dma_start(
                            ag_in[(rep, nm)][:].rearrange("(t p) n -> p t n",
                                                          p=128),
                            xn[:].rearrange("p (t n) -> p t n", t=OC))
                        if no_collective:
                            for r in range(NCORES):
                                nc.sync.dma_start(
                                    ag_out[(rep, nm)][r * HC:(r + 1) * HC, :],
                                    ag_in[(rep, nm)][:])
                        else:
                            nc.gpsimd.collective_compute(
                                "AllGather", mybir.AluOpType.bypass,
                                replica_groups=[list(range(NCORES))],
                                ins=[ag_in[(rep, nm)][:].opt()],
                                outs=[ag_out[(rep, nm)][:].opt()])

    nc.compile()
    return nc


def _get_nc():
    if "nc" not in _NC_CACHE:
        _NC_CACHE["nc"] = _build()
    return _NC_CACHE["nc"]


def _prep_inputs(inputs):
    f16 = np.float16
    x = np.asarray(inputs["x"], np.float32)
    a = np.asarray(inputs["a"], np.float32)
    base = {}
    base["xT0"] = np.ascontiguousarray(x.T).astype(f16)
    for (nm, F, C) in LAYERS:
        W = np.asarray(inputs["W" + nm], np.float32)   # [F, H, C]
        at = np.asarray(inputs["at" + nm], np.float32)  # [H, C]
        as_ = np.asarray(inputs["as" + nm], np.float32)
        wt = np.einsum("fhc,hc->fh", W, at)
        wcat = np.concatenate([W.reshape(F, H * C), wt], axis=1)
        base["Wc" + nm] = np.ascontiguousarray(wcat).astype(f16)
        base["Ws" + nm] = np.ascontiguousarray(
            np.einsum("fhc,hc->fh", W, as_)).astype(f16)
        base["b" + nm] = np.asarray(inputs["b" + nm], np.float32)
    maps = []
    xb = x.astype(np.float16).astype(np.float32)  # match device fp16
    for c in range(NCORES):
        m = dict(base)
        m["aT"] = np.ascontiguousarray(a[c * R:(c + 1) * R, :].T).astype(f16)
        m["xo0"] = np.ascontiguousarray(x[c * R:(c + 1) * R, :].T).astype(f16)
        xo = xb[c * R:(c + 1) * R, :]
        for blk, nm in ((0, "11"), (1, "21")):
            W = np.asarray(inputs["W" + nm], np.float32)
            as_ = np.asarray(inputs["as" + nm], np.float32)
            ws = np.einsum("fhc,hc->fh", W, as_)
            ws = ws.astype(np.float16).astype(np.float32)
            es = xo @ ws                       # [R, H]
            m["es1" if blk == 0 else "es2"] = np.ascontiguousarray(
                es.T.reshape(-1)).astype(np.float16)
        maps.append(m)
    return maps


def kernel(**inputs):
    nc = _get_nc()
    maps = _prep_inputs(inputs)
    res = run_bass_kernel_spmd(nc, maps, core_ids=list(range(NCORES)))
    out = np.zeros(768, np.float64)
    for c in range(NCORES):
        out += res.results[c]["pool"].astype(np.float64)
    return out.astype(np.float32)


if __name__ == "__main__":
    rng = np.random.default_rng(0)
    ins = {"x": rng.standard_normal((N, FIN)).astype(np.float32),
           "a": (rng.random((N, N)) < 0.01).astype(np.float32)}
    for (nm, F, C) in LAYERS:
        ins["W" + nm] = (rng.standard_normal((F, H, C)) / np.sqrt(F)).astype(np.float32)
        ins["as" + nm] = (rng.standard_normal((H, C)) * 0.1).astype(np.float32)
        ins["at" + nm] = (rng.standard_normal((H, C)) * 0.1).astype(np.float32)
        ins["b" + nm] = np.zeros(H * C, np.float32)
    out = kernel(**ins)
    print("kernel out[:8] =", out[:8])



# revision 22
# speedup vs baseline: 1.0301x; 1.0301x over previous
"""GAT (2-block, 3-layer) Trainium2 Bass kernel, 8-core SPMD.

Sharding: target-node rows (n) split across 8 cores (256 rows each).
Per layer, each core computes h = x @ W for ALL source nodes (needs the
full activation, obtained via AllGather), then row-local masked softmax
attention + aggregation for its 256 target rows.  The aggregation matmul
produces the TRANSPOSED activation [hc, n_own] which is exactly the
layout needed as lhsT for the next layer -- no transposes anywhere.
Final pooled vectors are partial-summed per core and reduced on host.

Self-contained: hardcodes all shapes; only needs /opt/trn_rl_repo.
"""
import sys
from contextlib import ExitStack

import numpy as np

sys.path.insert(0, "/opt/trn_rl_repo")

import concourse.bass as bass  # noqa: E402
import concourse.bacc as bacc  # noqa: E402
import concourse.tile as tile  # noqa: E402
from concourse import mybir  # noqa: E402
from concourse.bass_utils import run_bass_kernel_spmd  # noqa: E402

N = 2048
FIN = 128
H = 8
NCORES = 8
R = N // NCORES          # 256 target rows per core
MT = N // 128            # 16 source m-tiles
FP32 = mybir.dt.float32
FP16 = mybir.dt.float16

# (name, fan_in, C) ; blocks: 0 = layers *1x (C=64), 1 = *2x (C=32)
LAYERS = [("11", 128, 64), ("12", 512, 64), ("13", 512, 64),
          ("21", 128, 32), ("22", 256, 32), ("23", 256, 32)]
# emission order interleaves the two independent blocks so one block's
# compute hides the other's AllGather transition
SCHED = [("11", 128, 64, None), ("21", 128, 32, None),
         ("12", 512, 64, "11"), ("22", 256, 32, "21"),
         ("13", 512, 64, "12"), ("23", 256, 32, "22")]

_NC_CACHE = {}
DEBUG = False
# pointwise variant per (m_tile % len): A=ACT prelu+exp; B=DVE lrelu + ACT
# exp-half + DVE square-mask; G2=DVE s02 + GP max + ACT exp + GP mask;
# GM=A but mask on GP.  hcopy engine: a=ACT, v=DVE.
VARIANTS = ["GB", "A", "GB", "B", "GB", "GB", "A", "GB",
            "B", "GB", "GB", "GB", "A", "GB", "B", "B"]
HCOPY = {"A": "a", "B": "a", "G2": "a", "GM": "a", "GB": "a"}


def _build(repeat=1, no_collective=False):
    nc = bacc.Bacc("TRN2", target_bir_lowering=False, debug=False,
                   num_devices=NCORES)

    # ---------------- DRAM I/O ----------------
    xT0_d = nc.dram_tensor("xT0", [FIN, N], FP16, kind="ExternalInput")
    xo0_d = nc.dram_tensor("xo0", [FIN, R], FP16, kind="ExternalInput")
    aT_d = nc.dram_tensor("aT", [N, R], FP16, kind="ExternalInput")
    es1_d = nc.dram_tensor("es1", [N], FP16, kind="ExternalInput")
    es2_d = nc.dram_tensor("es2", [N], FP16, kind="ExternalInput")
    wc_d, ws_d, b_d = {}, {}, {}
    for (nm, F, C) in LAYERS:
        HC = H * C
        # Wcat = [W.reshape(F, HC) | Wt]  (Wt = einsum(W, at))
        wc_d[nm] = nc.dram_tensor(f"Wc{nm}", [F, HC + H], FP16,
                                  kind="ExternalInput")
        ws_d[nm] = nc.dram_tensor(f"Ws{nm}", [F, H], FP16, kind="ExternalInput")
        b_d[nm] = nc.dram_tensor(f"b{nm}", [HC], FP32, kind="ExternalInput")
    pool_d = nc.dram_tensor("pool", [768], FP32, kind="ExternalOutput")

    dbg_d = {}
    if DEBUG:
        for li, (nm, F, C) in enumerate(LAYERS):
            OC = (H * C) // 128
            dbg_d[nm] = nc.dram_tensor(f"dbg{nm}", [128, OC * R], FP16,
                                       kind="ExternalOutput")

    # internal DRAM: es scratch per layer + allgather buffers per transition
    es_scr, ag_in, ag_out = {}, {}, {}
    for rep in range(repeat):
        for li, (nm, F, C) in enumerate(LAYERS):
            key = (rep, nm)
            es_scr[key] = nc.dram_tensor(f"esscr{rep}_{nm}", [N], FP16,
                                         kind="Internal")
            if li % 3 != 2:
                HC = H * C
                ag_in[key] = nc.dram_tensor(f"agin{rep}_{nm}", [HC, R], FP16,
                                            kind="Internal")
                ag_out[key] = nc.dram_tensor(
                    f"agout{rep}_{nm}", [NCORES * HC, R], FP16,
                    kind="Internal", addr_space="Shared")

    with tile.TileContext(nc) as tc:
        with ExitStack() as ctx:
            pl = lambda **kw: ctx.enter_context(tc.tile_pool(**kw))  # noqa: E731
            constp = pl(name="const", bufs=1)
            wmp = pl(name="wm", bufs=2)
            wsmp = pl(name="wsm", bufs=2)
            xtb1p = pl(name="xtb1", bufs=1)
            xtb2p = pl(name="xtb2", bufs=1)
            hp = pl(name="hsb", bufs=6)
            esbp = pl(name="esb", bufs=1)
            essbp = pl(name="essb", bufs=2)
            etp = pl(name="etsb", bufs=4)
            sp = pl(name="s", bufs=4)
            up = pl(name="u", bufs=4)
            Pp = pl(name="P", bufs=4)
            pp = pl(name="p", bufs=6)
            xnp = pl(name="xn", bufs=2)
            dvp = pl(name="dv", bufs=8)
            dinvp = pl(name="dinv", bufs=2)
            dinvbp = pl(name="dinvb", bufs=2)
            poutp = pl(name="pout", bufs=1)
            php = pl(name="ph", bufs=2, space="PSUM")
            pep = pl(name="pe", bufs=1, space="PSUM")
            paggp = pl(name="pagg", bufs=4, space="PSUM")
            pDp = pl(name="pD", bufs=1, space="PSUM")

            # -------- prologue: resident inputs --------
            aT_sb = constp.tile([128, MT * R], FP16)          # [m | (mt, n)]
            aT_v = aT_d[:].rearrange("(t p) n -> p t n", p=128)
            _eng = [nc.gpsimd, nc.scalar, nc.gpsimd, nc.scalar]
            for j in range(4):
                _eng[j].dma_start(
                    aT_sb[:].rearrange("p (t n) -> p t n", t=MT)
                    [:, 4 * j:4 * j + 4, :],
                    aT_v[:, 4 * j:4 * j + 4, :])
            xT0_sb = constp.tile([128, N], FP16)
            nc.sync.dma_start(xT0_sb[:], xT0_d[:])
            xo0_sb = constp.tile([128, R], FP16)
            nc.sync.dma_start(xo0_sb[:], xo0_d[:])
            ones_sb = constp.tile([128, 1], FP16)
            nc.gpsimd.memset(ones_sb[:], 1.0)

            for rep in range(repeat):
                xn_prev = [None, None]   # per block: own transposed activation
                for li, (nm, F, C, prev) in enumerate(SCHED):
                    HC = H * C
                    FC = F // 128        # input chunks (of fan-in)
                    OC = HC // 128       # output chunks (of hc rows)
                    W2 = HC + H          # wcat width
                    blk = 0 if nm[0] == "1" else 1
                    lyr = int(nm[1]) - 1
                    fuse_et = (HC + H) <= 512   # block2: et inside h-matmul

                    # -------- weights --------
                    wc_sb = wmp.tile([128, FC * W2], FP16, tag="wm")
                    nc.gpsimd.dma_start(
                        wc_sb[:].rearrange("p (c d) -> p c d", c=FC),
                        wc_d[nm][:].rearrange("(c p) d -> p c d", p=128))
                    ws_sb = wsmp.tile([128, FC * H], FP16, tag="ws")
                    nc.gpsimd.dma_start(
                        ws_sb[:].rearrange("p (c d) -> p c d", c=FC),
                        ws_d[nm][:].rearrange("(c p) d -> p c d", p=128))
                    b_sb = wsmp.tile([128, OC], FP32, tag="b")
                    nc.gpsimd.dma_start(
                        b_sb[:], b_d[nm][:].rearrange("(c p) -> p c", p=128))

                    # -------- xT (all nodes, transposed) --------
                    if lyr == 0:
                        xT_sb = xT0_sb
                        xo_ap = xo0_sb
                    else:
                        pool_x = xtb1p if blk == 0 else xtb2p
                        xT_sb = pool_x.tile([128, FC * N], FP16, tag=f"xt{blk}")
                        gsrc = ag_out[(rep, prev)]
                        gv = gsrc[:].rearrange("(r c p) n -> p c r n",
                                               r=NCORES, p=128)
                        for fc in range(FC):
                            nc.sync.dma_start(
                                xT_sb[:, fc * N:(fc + 1) * N]
                                .rearrange("p (r n) -> p r n", r=NCORES),
                                gv[:, fc, :, :])
                        xo_ap = xn_prev[blk]

                    # -------- es chain --------
                    if lyr == 0:
                        es_src = es1_d if blk == 0 else es2_d
                    else:
                        es_src = es_scr[(rep, nm)]
                        pes = pep.tile([8, R], FP32, tag="pe")
                        for fc in range(FC):
                            nc.tensor.matmul(
                                pes[:], ws_sb[:, fc * H:(fc + 1) * H],
                                xo_ap[:, fc * R:(fc + 1) * R],
                                start=(fc == 0), stop=(fc == FC - 1))
                        es_sb = essbp.tile([8, R], FP16, tag="es")
                        nc.scalar.copy(es_sb[:], pes[:])
                        nc.gpsimd.dma_start(
                            es_src[:].rearrange("(h n) -> h n", h=8), es_sb[:])
                    esb = esbp.tile([128, N], FP16, tag="esb")
                    for j in range(2):
                        nc.sync.dma_start(
                            esb[:, j * 1024:(j + 1) * 1024],
                            es_src[j * 1024:(j + 1) * 1024][None, :]
                            .to_broadcast((128, 1024)))

                    # -------- aggregation psum (live across m loop) --------
                    agg_q = []
                    n_agg = 4 if C == 64 else 2
                    for _qi in range(n_agg):
                        agg_t = paggp.tile([128, 512], FP32, tag="agg")
                        agg_q.append(agg_t)
                    pD = pDp.tile([128, 512], FP32, tag="pD")

                    for i in range(MT):
                        # h (+ et fused for block2) for m-tile i
                        ph = php.tile([128, W2 if fuse_et else HC], FP32,
                                      tag="ph")
                        rw = W2 if fuse_et else HC
                        for fc in range(FC):
                            lhs = xT_sb[:, fc * N + i * 128:
                                        fc * N + (i + 1) * 128]
                            nc.tensor.matmul(
                                ph[:], lhs, wc_sb[:, fc * W2: fc * W2 + rw],
                                start=(fc == 0), stop=(fc == FC - 1))
                        if not fuse_et:
                            pet = pep.tile([128, H], FP32, tag="pe")
                            for fc in range(FC):
                                lhs = xT_sb[:, fc * N + i * 128:
                                            fc * N + (i + 1) * 128]
                                nc.tensor.matmul(
                                    pet[:], lhs,
                                    wc_sb[:, fc * W2 + HC: (fc + 1) * W2],
                                    start=(fc == 0), stop=(fc == FC - 1))
                        var = VARIANTS[i % len(VARIANTS)]
                        h_sb = hp.tile([128, HC], FP16, tag="h")
                        et_t = etp.tile([128, H], FP32, tag="et")
                        nc.scalar.copy(h_sb[:], ph[:, 0:HC])
                        if fuse_et:
                            nc.scalar.copy(et_t[:], ph[:, HC:W2])
                        else:
                            nc.scalar.copy(et_t[:], pet[:])
                        et_ap = et_t

                        # pointwise: p = aT * exp(lrelu_0.2(es + et))
                        # variant A (ACT-heavy): Prelu + Exp on ACT, mask DVE
                        # variant B (DVE-heavy): lrelu = max(s, .2s) on DVE,
                        #   exp(u/2) on ACT, then p = (A2*aT)*A2 on DVE
                        s_t = sp.tile([128, N], FP16, tag="s")
                        for h in range(H):
                            nc.vector.tensor_scalar(
                                s_t[:, h * R:(h + 1) * R],
                                esb[:, h * R:(h + 1) * R],
                                et_ap[:, h: h + 1], None,
                                mybir.AluOpType.add)
                        aT_ap = (aT_sb[:, i * R:(i + 1) * R][:, None, :]
                                 .to_broadcast((128, H, R)))
                        p_t = pp.tile([128, N], FP16, tag="p")
                        if var in ("A", "GM"):
                            u_t = up.tile([128, N], FP16, tag="u")
                            nc.scalar.activation(
                                u_t[:], s_t[:],
                                mybir.ActivationFunctionType.Prelu, alpha=0.2)
                            P_t = Pp.tile([128, N], FP16, tag="P")
                            nc.scalar.activation(
                                P_t[:], u_t[:],
                                mybir.ActivationFunctionType.Exp)
                            eng = nc.vector if var == "A" else nc.gpsimd
                            eng.tensor_tensor(
                                p_t[:].rearrange("p (h n) -> p h n", h=H),
                                P_t[:].rearrange("p (h n) -> p h n", h=H),
                                aT_ap, mybir.AluOpType.mult)
                        elif var == "GB":
                            # DVE lrelu, ACT exp, GP mask
                            s2_t = up.tile([128, N], FP16, tag="u")
                            nc.vector.tensor_scalar(
                                s2_t[:], s_t[:], 0.2, None,
                                mybir.AluOpType.mult)
                            u2_t = up.tile([128, N], FP16, tag="u")
                            nc.vector.tensor_tensor(
                                u2_t[:], s_t[:], s2_t[:],
                                mybir.AluOpType.max)
                            P_t = Pp.tile([128, N], FP16, tag="P")
                            nc.scalar.activation(
                                P_t[:], u2_t[:],
                                mybir.ActivationFunctionType.Exp)
                            nc.gpsimd.tensor_tensor(
                                p_t[:].rearrange("p (h n) -> p h n", h=H),
                                P_t[:].rearrange("p (h n) -> p h n", h=H),
                                aT_ap, mybir.AluOpType.mult)
                        elif var == "B":
                            # lrelu on DVE (max(s, .2s)), exp on ACT, mask DVE
                            s2_t = up.tile([128, N], FP16, tag="u")
                            nc.vector.tensor_scalar(
                                s2_t[:], s_t[:], 0.2, None,
                                mybir.AluOpType.mult)
                            u2_t = up.tile([128, N], FP16, tag="u")
                            nc.vector.tensor_tensor(
                                u2_t[:], s_t[:], s2_t[:],
                                mybir.AluOpType.max)
                            P_t = Pp.tile([128, N], FP16, tag="P")
                            nc.scalar.activation(
                                P_t[:], u2_t[:],
                                mybir.ActivationFunctionType.Exp)
                            nc.vector.tensor_tensor(
                                p_t[:].rearrange("p (h n) -> p h n", h=H),
                                P_t[:].rearrange("p (h n) -> p h n", h=H),
                                aT_ap, mybir.AluOpType.mult)
                        else:  # G2: DVE s02, GP max, ACT exp, GP mask
                            s2_t = up.tile([128, N], FP16, tag="u")
                            nc.vector.tensor_scalar(
                                s2_t[:], s_t[:], 0.2, None,
                                mybir.AluOpType.mult)
                            nc.gpsimd.tensor_tensor(
                                s_t[:], s_t[:], s2_t[:],
                                mybir.AluOpType.max)
                            P_t = Pp.tile([128, N], FP16, tag="P")
                            nc.scalar.activation(
                                P_t[:], s_t[:],
                                mybir.ActivationFunctionType.Exp)
                            nc.gpsimd.tensor_tensor(
                                p_t[:].rearrange("p (h n) -> p h n", h=H),
                                P_t[:].rearrange("p (h n) -> p h n", h=H),
                                aT_ap, mybir.AluOpType.mult)

                        # aggregation: 2 heads per matmul ([128, 512] rhs).
                        # One accumulation group per (partition-range, bank).
                        for j in range(4):
                            lhsT = h_sb[:, j * 2 * C:(j + 1) * 2 * C]
                            rhs = p_t[:, j * 512:(j + 1) * 512]
                            if C == 64:
                                nc.tensor.matmul(
                                    agg_q[j][:, :], lhsT, rhs,
                                    start=(i == 0), stop=(i == MT - 1),
                                    tile_position=(0, 0))
                            else:
                                pb = (j % 2) * 64
                                nc.tensor.matmul(
                                    agg_q[j // 2][pb:pb + 64, :], lhsT, rhs,
                                    start=(i == 0), stop=(i == MT - 1),
                                    tile_position=(0, pb),
                                    skip_group_check=(pb > 0))
                        for j in range(4):
                            nc.tensor.matmul(
                                pD[32 * j:32 * j + 1, :],
                                ones_sb[:],
                                p_t[:, j * 512:(j + 1) * 512],
                                start=(i == 0), stop=(i == MT - 1),
                                tile_position=(0, 32 * j),
                                skip_group_check=(j > 0))

                    # -------- finalize: alpha-normalize + bias + relu ------
                    dinv = dinvp.tile([1, N], FP32, tag="dinv")
                    for j in range(4):
                        nc.vector.reciprocal(dinv[0:1, j * 512:(j + 1) * 512],
                                             pD[32 * j:32 * j + 1, :])
                    dinvb = dinvbp.tile([128, N], FP32, tag="dinvb")
                    nc.gpsimd.partition_broadcast(dinvb[:], dinv[0:1, :])
                    xn = xnp.tile([128, OC * R], FP16, tag=f"xn{blk}")
                    hpc = 128 // C  # heads per 128-row chunk
                    for t in range(OC):
                        for k in range(hpc):
                            pb, h = k * C, t * hpc + k
                            fo = (k % 2) * 256
                            # b1: chunk t = pair tile t; b2: tile t, see map
                            src = agg_q[t][pb:pb + C, fo:fo + R]
                            dv = dvp.tile([128, R], FP32, tag="dv")
                            nc.vector.tensor_tensor(
                                dv[pb:pb + C, :], src,
                                dinvb[pb:pb + C, h * R:(h + 1) * R],
                                mybir.AluOpType.mult)
                            nc.scalar.activation(
                                xn[pb:pb + C, t * R:(t + 1) * R],
                                dv[pb:pb + C, :],
                                mybir.ActivationFunctionType.Relu,
                                bias=b_sb[pb:pb + C, t:t + 1])

                    if DEBUG:
                        nc.sync.dma_start(dbg_d[nm][:, 0:OC * R], xn[:])
                    if lyr == 2:
                        # global pool: partial sum over own 256 rows
                        po = poutp.tile([128, OC], FP32, tag=f"po{blk}")
                        for t in range(OC):
                            nc.vector.tensor_reduce(
                                po[:, t:t + 1], xn[:, t * R:(t + 1) * R],
                                axis=mybir.AxisListType.X,
                                op=mybir.AluOpType.add)
                        off = 0 if blk == 0 else 512
                        nc.sync.dma_start(
                            pool_d[off:off + HC].rearrange("(c p) -> p c",
                                                           p=128),
                            po[:])
                    else:
                        xn_prev[blk] = xn
                        # on SP queue: the next layer's esb broadcasts queue
                        # behind this write, so its pointwise starts in sync
                        # with this AllGather and hides it
                        nc.sync.dma_start(
                            ag_in[(rep, nm)][:].rearrange("(t p) n -> p t n",
                                                          p=128),
                            xn[:].rearrange("p (t n) -> p t n", t=OC))
                        if no_collective:
                            for r in range(NCORES):
                                nc.sync.dma_start(
                                    ag_out[(rep, nm)][r * HC:(r + 1) * HC, :],
                                    ag_in[(rep, nm)][:])
                        else:
                            nc.gpsimd.collective_compute(
                                "AllGather", mybir.AluOpType.bypass,
                                replica_groups=[list(range(NCORES))],
                                ins=[ag_in[(rep, nm)][:].opt()],
                                outs=[ag_out[(rep, nm)][:].opt()])

    nc.compile()
    return nc


def _get_nc():
    if "nc" not in _NC_CACHE:
        _NC_CACHE["nc"] = _build()
    return _NC_CACHE["nc"]


def _prep_inputs(inputs):
    f16 = np.float16
    x = np.asarray(inputs["x"], np.float32)
    a = np.asarray(inputs["a"], np.float32)
    base = {}
    base["xT0"] = np.ascontiguousarray(x.T).astype(f16)
    for (nm, F, C) in LAYERS:
        W = np.asarray(inputs["W" + nm], np.float32)   # [F, H, C]
        at = np.asarray(inputs["at" + nm], np.float32)  # [H, C]
        as_ = np.asarray(inputs["as" + nm], np.float32)
        wt = np.einsum("fhc,hc->fh", W, at)
        wcat = np.concatenate([W.reshape(F, H * C), wt], axis=1)
        base["Wc" + nm] = np.ascontiguousarray(wcat).astype(f16)
        base["Ws" + nm] = np.ascontiguousarray(
            np.einsum("fhc,hc->fh", W, as_)).astype(f16)
        base["b" + nm] = np.asarray(inputs["b" + nm], np.float32)
    maps = []
    xb = x.astype(np.float16).astype(np.float32)  # match device fp16
    for c in range(NCORES):
        m = dict(base)
        m["aT"] = np.ascontiguousarray(a[c * R:(c + 1) * R, :].T).astype(f16)
        m["xo0"] = np.ascontiguousarray(x[c * R:(c + 1) * R, :].T).astype(f16)
        xo = xb[c * R:(c + 1) * R, :]
        for blk, nm in ((0, "11"), (1, "21")):
            W = np.asarray(inputs["W" + nm], np.float32)
            as_ = np.asarray(inputs["as" + nm], np.float32)
            ws = np.einsum("fhc,hc->fh", W, as_)
            ws = ws.astype(np.float16).astype(np.float32)
            es = xo @ ws                       # [R, H]
            m["es1" if blk == 0 else "es2"] = np.ascontiguousarray(
                es.T.reshape(-1)).astype(np.float16)
        maps.append(m)
    return maps


def kernel(**inputs):
    nc = _get_nc()
    maps = _prep_inputs(inputs)
    res = run_bass_kernel_spmd(nc, maps, core_ids=list(range(NCORES)))
    out = np.zeros(768, np.float64)
    for c in range(NCORES):
        out += res.results[c]["pool"].astype(np.float64)
    return out.astype(np.float32)


if __name__ == "__main__":
    rng = np.random.default_rng(0)
    ins = {"x": rng.standard_normal((N, FIN)).astype(np.float32),
           "a": (rng.random((N, N)) < 0.01).astype(np.float32)}
    for (nm, F, C) in LAYERS:
        ins["W" + nm] = (rng.standard_normal((F, H, C)) / np.sqrt(F)).astype(np.float32)
        ins["as" + nm] = (rng.standard_normal((H, C)) * 0.1).astype(np.float32)
        ins["at" + nm] = (rng.standard_normal((H, C)) * 0.1).astype(np.float32)
        ins["b" + nm] = np.zeros(H * C, np.float32)
    out = kernel(**ins)
    print("kernel out[:8] =", out[:8])



# revision 28
# speedup vs baseline: 1.0479x; 1.0173x over previous
"""GAT (2-block, 3-layer) Trainium2 Bass kernel, 8-core SPMD.

Sharding: target-node rows (n) split across 8 cores (256 rows each).
Per layer, each core computes h = x @ W for ALL source nodes (needs the
full activation, obtained via AllGather), then row-local masked softmax
attention + aggregation for its 256 target rows.  The aggregation matmul
produces the TRANSPOSED activation [hc, n_own] which is exactly the
layout needed as lhsT for the next layer -- no transposes anywhere.
Final pooled vectors are partial-summed per core and reduced on host.

Self-contained: hardcodes all shapes; only needs /opt/trn_rl_repo.
"""
import sys
from contextlib import ExitStack

import numpy as np

sys.path.insert(0, "/opt/trn_rl_repo")

import concourse.bass as bass  # noqa: E402
import concourse.bacc as bacc  # noqa: E402
import concourse.tile as tile  # noqa: E402
from concourse import mybir  # noqa: E402
from concourse.bass_utils import run_bass_kernel_spmd  # noqa: E402

N = 2048
FIN = 128
H = 8
NCORES = 8
R = N // NCORES          # 256 target rows per core
MT = N // 128            # 16 source m-tiles
FP32 = mybir.dt.float32
FP16 = mybir.dt.float16

# (name, fan_in, C) ; blocks: 0 = layers *1x (C=64), 1 = *2x (C=32)
LAYERS = [("11", 128, 64), ("12", 512, 64), ("13", 512, 64),
          ("21", 128, 32), ("22", 256, 32), ("23", 256, 32)]
# emission order interleaves the two independent blocks so one block's
# compute hides the other's AllGather transition
SCHED = [("11", 128, 64, None), ("21", 128, 32, None),
         ("12", 512, 64, "11"), ("22", 256, 32, "21"),
         ("13", 512, 64, "12"), ("23", 256, 32, "22")]

_NC_CACHE = {}
DEBUG = False
# pointwise variant per (m_tile % len): A=ACT prelu+exp; B=DVE lrelu + ACT
# exp-half + DVE square-mask; G2=DVE s02 + GP max + ACT exp + GP mask;
# GM=A but mask on GP.  hcopy engine: a=ACT, v=DVE.
VARIANTS = ["GB", "A", "GB", "B", "GB", "GB", "A", "GB",
            "B", "GB", "GB", "GB", "A", "GB", "B", "B"]
HCOPY = {"A": "a", "B": "a", "G2": "a", "GM": "a", "GB": "a"}


def _build(repeat=1, no_collective=False):
    nc = bacc.Bacc("TRN2", target_bir_lowering=False, debug=False,
                   num_devices=NCORES)

    # ---------------- DRAM I/O ----------------
    xT0_d = nc.dram_tensor("xT0", [FIN, N], FP16, kind="ExternalInput")
    xo0_d = nc.dram_tensor("xo0", [FIN, R], FP16, kind="ExternalInput")
    aT_d = nc.dram_tensor("aT", [N, R], FP16, kind="ExternalInput")
    es1_d = nc.dram_tensor("es1", [N], FP16, kind="ExternalInput")
    es2_d = nc.dram_tensor("es2", [N], FP16, kind="ExternalInput")
    wc_d, ws_d, b_d = {}, {}, {}
    for (nm, F, C) in LAYERS:
        HC = H * C
        # Wcat = [W.reshape(F, HC) | Wt]  (Wt = einsum(W, at))
        wc_d[nm] = nc.dram_tensor(f"Wc{nm}", [F, HC + H], FP16,
                                  kind="ExternalInput")
        ws_d[nm] = nc.dram_tensor(f"Ws{nm}", [F, H], FP16, kind="ExternalInput")
        b_d[nm] = nc.dram_tensor(f"b{nm}", [HC], FP32, kind="ExternalInput")
    pool_d = nc.dram_tensor("pool", [768], FP32, kind="ExternalOutput")

    dbg_d = {}
    if DEBUG:
        for li, (nm, F, C) in enumerate(LAYERS):
            OC = (H * C) // 128
            dbg_d[nm] = nc.dram_tensor(f"dbg{nm}", [128, OC * R], FP16,
                                       kind="ExternalOutput")

    # internal DRAM: es scratch per layer + allgather buffers per transition
    es_scr, ag_in, ag_out = {}, {}, {}
    for rep in range(repeat):
        for li, (nm, F, C) in enumerate(LAYERS):
            key = (rep, nm)
            es_scr[key] = nc.dram_tensor(f"esscr{rep}_{nm}", [N], FP16,
                                         kind="Internal")
            if li % 3 != 2:
                HC = H * C
                ag_in[key] = nc.dram_tensor(f"agin{rep}_{nm}", [HC, R], FP16,
                                            kind="Internal")
                ag_out[key] = nc.dram_tensor(
                    f"agout{rep}_{nm}", [NCORES * HC, R], FP16,
                    kind="Internal", addr_space="Shared")

    with tile.TileContext(nc) as tc:
        with ExitStack() as ctx:
            pl = lambda **kw: ctx.enter_context(tc.tile_pool(**kw))  # noqa: E731
            constp = pl(name="const", bufs=1)
            wmp = pl(name="wm", bufs=2)
            wsmp = pl(name="wsm", bufs=2)
            xtb1p = pl(name="xtb1", bufs=1)
            xtb2p = pl(name="xtb2", bufs=1)
            hp = pl(name="hsb", bufs=6)
            esbp = pl(name="esb", bufs=1)
            essbp = pl(name="essb", bufs=2)
            etp = pl(name="etsb", bufs=4)
            sp = pl(name="s", bufs=4)
            up = pl(name="u", bufs=4)
            Pp = pl(name="P", bufs=4)
            pp = pl(name="p", bufs=6)
            xnp = pl(name="xn", bufs=2)
            dvp = pl(name="dv", bufs=8)
            dinvp = pl(name="dinv", bufs=2)
            dinvbp = pl(name="dinvb", bufs=2)
            poutp = pl(name="pout", bufs=1)
            php = pl(name="ph", bufs=2, space="PSUM")
            pep = pl(name="pe", bufs=1, space="PSUM")
            paggp = pl(name="pagg", bufs=4, space="PSUM")
            pDp = pl(name="pD", bufs=1, space="PSUM")

            # -------- prologue: resident inputs --------
            aT_sb = constp.tile([128, MT * R], FP16)          # [m | (mt, n)]
            aT_v = aT_d[:].rearrange("(t p) n -> p t n", p=128)
            _eng = [nc.gpsimd, nc.scalar, nc.gpsimd, nc.scalar]
            for j in range(4):
                _eng[j].dma_start(
                    aT_sb[:].rearrange("p (t n) -> p t n", t=MT)
                    [:, 4 * j:4 * j + 4, :],
                    aT_v[:, 4 * j:4 * j + 4, :])
            xT0_sb = constp.tile([128, N], FP16)
            nc.sync.dma_start(xT0_sb[:], xT0_d[:])
            xo0_sb = constp.tile([128, R], FP16)
            nc.sync.dma_start(xo0_sb[:], xo0_d[:])
            ones_sb = constp.tile([128, 1], FP16)
            nc.gpsimd.memset(ones_sb[:], 1.0)

            for rep in range(repeat):
                xn_prev = [None, None]   # per block: own transposed activation
                for li, (nm, F, C, prev) in enumerate(SCHED):
                    HC = H * C
                    FC = F // 128        # input chunks (of fan-in)
                    OC = HC // 128       # output chunks (of hc rows)
                    W2 = HC + H          # wcat width
                    blk = 0 if nm[0] == "1" else 1
                    lyr = int(nm[1]) - 1
                    fuse_et = (HC + H) <= 512   # block2: et inside h-matmul

                    # -------- weights --------
                    wc_sb = wmp.tile([128, FC * W2], FP16, tag="wm")
                    nc.gpsimd.dma_start(
                        wc_sb[:].rearrange("p (c d) -> p c d", c=FC),
                        wc_d[nm][:].rearrange("(c p) d -> p c d", p=128))
                    ws_sb = wsmp.tile([128, FC * H], FP16, tag="ws")
                    nc.gpsimd.dma_start(
                        ws_sb[:].rearrange("p (c d) -> p c d", c=FC),
                        ws_d[nm][:].rearrange("(c p) d -> p c d", p=128))
                    b_sb = wsmp.tile([128, OC], FP32, tag="b")
                    nc.gpsimd.dma_start(
                        b_sb[:], b_d[nm][:].rearrange("(c p) -> p c", p=128))

                    # -------- xT (all nodes, transposed) --------
                    if lyr == 0:
                        xT_sb = xT0_sb
                        xo_ap = xo0_sb
                    else:
                        pool_x = xtb1p if blk == 0 else xtb2p
                        xT_sb = pool_x.tile([128, FC * N], FP16, tag=f"xt{blk}")
                        gsrc = ag_out[(rep, prev)]
                        gv = gsrc[:].rearrange("(r c p) n -> p c r n",
                                               r=NCORES, p=128)
                        for fc in range(FC):
                            nc.sync.dma_start(
                                xT_sb[:, fc * N:(fc + 1) * N]
                                .rearrange("p (r n) -> p r n", r=NCORES),
                                gv[:, fc, :, :])
                        xo_ap = xn_prev[blk]

                    # -------- es chain --------
                    if lyr == 0:
                        es_src = es1_d if blk == 0 else es2_d
                    else:
                        es_src = es_scr[(rep, nm)]
                        pes = pep.tile([8, R], FP32, tag="pe")
                        for fc in range(FC):
                            nc.tensor.matmul(
                                pes[:], ws_sb[:, fc * H:(fc + 1) * H],
                                xo_ap[:, fc * R:(fc + 1) * R],
                                start=(fc == 0), stop=(fc == FC - 1))
                        es_sb = essbp.tile([8, R], FP16, tag="es")
                        nc.scalar.copy(es_sb[:], pes[:])
                        nc.gpsimd.dma_start(
                            es_src[:].rearrange("(h n) -> h n", h=8), es_sb[:])
                    esb = esbp.tile([128, N], FP16, tag="esb")
                    for j in range(2):
                        nc.sync.dma_start(
                            esb[:, j * 1024:(j + 1) * 1024],
                            es_src[j * 1024:(j + 1) * 1024][None, :]
                            .to_broadcast((128, 1024)))

                    # -------- aggregation psum (live across m loop) --------
                    agg_q = []
                    n_agg = 4 if C == 64 else 2
                    for _qi in range(n_agg):
                        agg_t = paggp.tile([128, 512], FP32, tag="agg")
                        agg_q.append(agg_t)
                    pD = pDp.tile([128, 512], FP32, tag="pD")

                    for i in range(MT):
                        # h (+ et fused for block2) for m-tile i
                        ph = php.tile([128, W2 if fuse_et else HC], FP32,
                                      tag="ph")
                        rw = W2 if fuse_et else HC
                        for fc in range(FC):
                            lhs = xT_sb[:, fc * N + i * 128:
                                        fc * N + (i + 1) * 128]
                            nc.tensor.matmul(
                                ph[:], lhs, wc_sb[:, fc * W2: fc * W2 + rw],
                                start=(fc == 0), stop=(fc == FC - 1))
                        if not fuse_et:
                            pet = pep.tile([128, H], FP32, tag="pe")
                            for fc in range(FC):
                                lhs = xT_sb[:, fc * N + i * 128:
                                            fc * N + (i + 1) * 128]
                                nc.tensor.matmul(
                                    pet[:], lhs,
                                    wc_sb[:, fc * W2 + HC: (fc + 1) * W2],
                                    start=(fc == 0), stop=(fc == FC - 1))
                        var = VARIANTS[i % len(VARIANTS)]
                        h_sb = hp.tile([128, HC], FP16, tag="h")
                        et_t = etp.tile([128, H], FP32, tag="et")
                        nc.scalar.copy(h_sb[:], ph[:, 0:HC])
                        if fuse_et:
                            nc.scalar.copy(et_t[:], ph[:, HC:W2])
                        else:
                            nc.scalar.copy(et_t[:], pet[:])
                        et_ap = et_t

                        # pointwise: p = aT * exp(lrelu_0.2(es + et))
                        # variant A (ACT-heavy): Prelu + Exp on ACT, mask DVE
                        # variant B (DVE-heavy): lrelu = max(s, .2s) on DVE,
                        #   exp(u/2) on ACT, then p = (A2*aT)*A2 on DVE
                        s_t = sp.tile([128, N], FP16, tag="s")
                        for h in range(H):
                            nc.vector.tensor_scalar(
                                s_t[:, h * R:(h + 1) * R],
                                esb[:, h * R:(h + 1) * R],
                                et_ap[:, h: h + 1], None,
                                mybir.AluOpType.add)
                        aT_ap = (aT_sb[:, i * R:(i + 1) * R][:, None, :]
                                 .to_broadcast((128, H, R)))
                        p_t = pp.tile([128, N], FP16, tag="p")
                        if var in ("A", "GM"):
                            u_t = up.tile([128, N], FP16, tag="u")
                            nc.scalar.activation(
                                u_t[:], s_t[:],
                                mybir.ActivationFunctionType.Prelu, alpha=0.2)
                            P_t = Pp.tile([128, N], FP16, tag="P")
                            nc.scalar.activation(
                                P_t[:], u_t[:],
                                mybir.ActivationFunctionType.Exp)
                            eng = nc.vector if var == "A" else nc.gpsimd
                            eng.tensor_tensor(
                                p_t[:].rearrange("p (h n) -> p h n", h=H),
                                P_t[:].rearrange("p (h n) -> p h n", h=H),
                                aT_ap, mybir.AluOpType.mult)
                        elif var == "GB":
                            # DVE lrelu, ACT exp, GP mask
                            s2_t = up.tile([128, N], FP16, tag="u")
                            nc.vector.tensor_scalar(
                                s2_t[:], s_t[:], 0.2, None,
                                mybir.AluOpType.mult)
                            u2_t = up.tile([128, N], FP16, tag="u")
                            nc.vector.tensor_tensor(
                                u2_t[:], s_t[:], s2_t[:],
                                mybir.AluOpType.max)
                            P_t = Pp.tile([128, N], FP16, tag="P")
                            nc.scalar.activation(
                                P_t[:], u2_t[:],
                                mybir.ActivationFunctionType.Exp)
                            nc.gpsimd.tensor_tensor(
                                p_t[:].rearrange("p (h n) -> p h n", h=H),
                                P_t[:].rearrange("p (h n) -> p h n", h=H),
                                aT_ap, mybir.AluOpType.mult)
                        elif var == "B":
                            # lrelu on DVE (max(s, .2s)), exp on ACT, mask DVE
                            s2_t = up.tile([128, N], FP16, tag="u")
                            nc.vector.tensor_scalar(
                                s2_t[:], s_t[:], 0.2, None,
                                mybir.AluOpType.mult)
                            u2_t = up.tile([128, N], FP16, tag="u")
                            nc.vector.tensor_tensor(
                                u2_t[:], s_t[:], s2_t[:],
                                mybir.AluOpType.max)
                            P_t = Pp.tile([128, N], FP16, tag="P")
                            nc.scalar.activation(
                                P_t[:], u2_t[:],
                                mybir.ActivationFunctionType.Exp)
                            nc.vector.tensor_tensor(
                                p_t[:].rearrange("p (h n) -> p h n", h=H),
                                P_t[:].rearrange("p (h n) -> p h n", h=H),
                                aT_ap, mybir.AluOpType.mult)
                        else:  # G2: DVE s02, GP max, ACT exp, GP mask
                            s2_t = up.tile([128, N], FP16, tag="u")
                            nc.vector.tensor_scalar(
                                s2_t[:], s_t[:], 0.2, None,
                                mybir.AluOpType.mult)
                            nc.gpsimd.tensor_tensor(
                                s_t[:], s_t[:], s2_t[:],
                                mybir.AluOpType.max)
                            P_t = Pp.tile([128, N], FP16, tag="P")
                            nc.scalar.activation(
                                P_t[:], s_t[:],
                                mybir.ActivationFunctionType.Exp)
                            nc.gpsimd.tensor_tensor(
                                p_t[:].rearrange("p (h n) -> p h n", h=H),
                                P_t[:].rearrange("p (h n) -> p h n", h=H),
                                aT_ap, mybir.AluOpType.mult)

                        # aggregation: 2 heads per matmul ([128, 512] rhs).
                        # One accumulation group per (partition-range, bank).
                        for j in range(4):
                            lhsT = h_sb[:, j * 2 * C:(j + 1) * 2 * C]
                            rhs = p_t[:, j * 512:(j + 1) * 512]
                            if C == 64:
                                nc.tensor.matmul(
                                    agg_q[j][:, :], lhsT, rhs,
                                    start=(i == 0), stop=(i == MT - 1),
                                    tile_position=(0, 0))
                            else:
                                pb = (j % 2) * 64
                                nc.tensor.matmul(
                                    agg_q[j // 2][pb:pb + 64, :], lhsT, rhs,
                                    start=(i == 0), stop=(i == MT - 1),
                                    tile_position=(0, pb),
                                    skip_group_check=(pb > 0))
                        for j in range(4):
                            nc.tensor.matmul(
                                pD[32 * j:32 * j + 1, :],
                                ones_sb[:],
                                p_t[:, j * 512:(j + 1) * 512],
                                start=(i == 0), stop=(i == MT - 1),
                                tile_position=(0, 32 * j),
                                skip_group_check=(j > 0))

                    # -------- finalize: alpha-normalize + bias + relu ------
                    # pipelined per head-pair: recip chunk (DVE) -> broadcast
                    # chunk (GP, from partition 0) -> normalize+relu, so the
                    # ag_in payload is ready ~8us earlier than monolithic
                    # stages and the AllGather starts sooner
                    dinv = dinvp.tile([1, N], FP32, tag="dinv")
                    dinvb = dinvbp.tile([128, N], FP32, tag="dinvb")
                    xn = xnp.tile([128, OC * R], FP16, tag=f"xn{blk}")
                    hpc = 128 // C  # heads per 128-row chunk
                    for j in range(4):
                        sl = slice(j * 512, (j + 1) * 512)
                        nc.vector.reciprocal(dinv[0:1, sl],
                                             pD[32 * j:32 * j + 1, :])
                        nc.gpsimd.partition_broadcast(dinvb[:, sl],
                                                      dinv[0:1, sl])
                        for h in (2 * j, 2 * j + 1):
                            t, k = divmod(h, hpc)
                            pb = k * C
                            fo = (k % 2) * 256
                            # b1: chunk t = pair tile t; b2: tile t, see map
                            src = agg_q[t][pb:pb + C, fo:fo + R]
                            dv = dvp.tile([128, R], FP32, tag="dv")
                            nc.vector.tensor_tensor(
                                dv[pb:pb + C, :], src,
                                dinvb[pb:pb + C, h * R:(h + 1) * R],
                                mybir.AluOpType.mult)
                            nc.scalar.activation(
                                xn[pb:pb + C, t * R:(t + 1) * R],
                                dv[pb:pb + C, :],
                                mybir.ActivationFunctionType.Relu,
                                bias=b_sb[pb:pb + C, t:t + 1])

                    if DEBUG:
                        nc.sync.dma_start(dbg_d[nm][:, 0:OC * R], xn[:])
                    if lyr == 2:
                        # global pool: partial sum over own 256 rows
                        po = poutp.tile([128, OC], FP32, tag=f"po{blk}")
                        for t in range(OC):
                            nc.vector.tensor_reduce(
                                po[:, t:t + 1], xn[:, t * R:(t + 1) * R],
                                axis=mybir.AxisListType.X,
                                op=mybir.AluOpType.add)
                        off = 0 if blk == 0 else 512
                        nc.sync.dma_start(
                            pool_d[off:off + HC].rearrange("(c p) -> p c",
                                                           p=128),
                            po[:])
                    else:
                        xn_prev[blk] = xn
                        # per-chunk DMAs pipeline the payload write with the
                        # finalize, so the AllGather dispatches sooner
                        agv = ag_in[(rep, nm)][:].rearrange(
                            "(t p) n -> t p n", p=128)
                        for tt in range(OC):
                            nc.sync.dma_start(
                                agv[tt], xn[:, tt * R:(tt + 1) * R])
                        if no_collective:
                            for r in range(NCORES):
                                nc.sync.dma_start(
                                    ag_out[(rep, nm)][r * HC:(r + 1) * HC, :],
                                    ag_in[(rep, nm)][:])
                        else:
                            nc.gpsimd.collective_compute(
                                "AllGather", mybir.AluOpType.bypass,
                                replica_groups=[list(range(NCORES))],
                                ins=[ag_in[(rep, nm)][:].opt()],
                                outs=[ag_out[(rep, nm)][:].opt()])

    nc.compile()
    return nc


def _get_nc():
    if "nc" not in _NC_CACHE:
        _NC_CACHE["nc"] = _build()
    return _NC_CACHE["nc"]


def _prep_inputs(inputs):
    f16 = np.float16
    x = np.asarray(inputs["x"], np.float32)
    a = np.asarray(inputs["a"], np.float32)
    base = {}
    base["xT0"] = np.ascontiguousarray(x.T).astype(f16)
    for (nm, F, C) in LAYERS:
        W = np.asarray(inputs["W" + nm], np.float32)   # [F, H, C]
        at = np.asarray(inputs["at" + nm], np.float32)  # [H, C]
        as_ = np.asarray(inputs["as" + nm], np.float32)
        wt = np.einsum("fhc,hc->fh", W, at)
        wcat = np.concatenate([W.reshape(F, H * C), wt], axis=1)
        base["Wc" + nm] = np.ascontiguousarray(wcat).astype(f16)
        base["Ws" + nm] = np.ascontiguousarray(
            np.einsum("fhc,hc->fh", W, as_)).astype(f16)
        base["b" + nm] = np.asarray(inputs["b" + nm], np.float32)
    maps = []
    xb = x.astype(np.float16).astype(np.float32)  # match device fp16
    for c in range(NCORES):
        m = dict(base)
        m["aT"] = np.ascontiguousarray(a[c * R:(c + 1) * R, :].T).astype(f16)
        m["xo0"] = np.ascontiguousarray(x[c * R:(c + 1) * R, :].T).astype(f16)
        xo = xb[c * R:(c + 1) * R, :]
        for blk, nm in ((0, "11"), (1, "21")):
            W = np.asarray(inputs["W" + nm], np.float32)
            as_ = np.asarray(inputs["as" + nm], np.float32)
            ws = np.einsum("fhc,hc->fh", W, as_)
            ws = ws.astype(np.float16).astype(np.float32)
            es = xo @ ws                       # [R, H]
            m["es1" if blk == 0 else "es2"] = np.ascontiguousarray(
                es.T.reshape(-1)).astype(np.float16)
        maps.append(m)
    return maps


def kernel(**inputs):
    nc = _get_nc()
    maps = _prep_inputs(inputs)
    res = run_bass_kernel_spmd(nc, maps, core_ids=list(range(NCORES)))
    out = np.zeros(768, np.float64)
    for c in range(NCORES):
        out += res.results[c]["pool"].astype(np.float64)
    return out.astype(np.float32)


if __name__ == "__main__":
    rng = np.random.default_rng(0)
    ins = {"x": rng.standard_normal((N, FIN)).astype(np.float32),
           "a": (rng.random((N, N)) < 0.01).astype(np.float32)}
    for (nm, F, C) in LAYERS:
        ins["W" + nm] = (rng.standard_normal((F, H, C)) / np.sqrt(F)).astype(np.float32)
        ins["as" + nm] = (rng.standard_normal((H, C)) * 0.1).astype(np.float32)
        ins["at" + nm] = (rng.standard_normal((H, C)) * 0.1).astype(np.float32)
        ins["b" + nm] = np.zeros(H * C, np.float32)
    out = kernel(**ins)
    print("kernel out[:8] =", out[:8])



# revision 43
# speedup vs baseline: 1.2470x; 1.1900x over previous
"""GAT (2-block, 3-layer) Trainium2 Bass kernel, 8-core SPMD.

Sharding: target-node rows (n) split across 8 cores (256 rows each).
Per layer, each core computes h = x @ W for ALL source nodes (needs the
full activation, obtained via AllGather), then row-local masked softmax
attention + aggregation for its 256 target rows.  The aggregation matmul
produces the TRANSPOSED activation [hc, n_own] which is exactly the
layout needed as lhsT for the next layer -- no transposes anywhere.
Final pooled vectors are partial-summed per core and reduced on host.

Self-contained: hardcodes all shapes; only needs /opt/trn_rl_repo.
"""
import sys
from contextlib import ExitStack

import numpy as np

sys.path.insert(0, "/opt/trn_rl_repo")

import concourse.bass as bass  # noqa: E402
import concourse.bacc as bacc  # noqa: E402
import concourse.tile as tile  # noqa: E402
from concourse import mybir  # noqa: E402
from concourse.bass_utils import run_bass_kernel_spmd  # noqa: E402

N = 2048
FIN = 128
H = 8
NCORES = 8
R = N // NCORES          # 256 target rows per core
MT = N // 128            # 16 source m-tiles
FP32 = mybir.dt.float32
FP16 = mybir.dt.float16

# (name, fan_in, C) ; blocks: 0 = layers *1x (C=64), 1 = *2x (C=32)
LAYERS = [("11", 128, 64), ("12", 512, 64), ("13", 512, 64),
          ("21", 128, 32), ("22", 256, 32), ("23", 256, 32)]
# emission order interleaves the two independent blocks so one block's
# compute hides the other's AllGather transition
SCHED = [("11", 128, 64, None), ("21", 128, 32, None),
         ("12", 512, 64, "11"), ("22", 256, 32, "21"),
         ("13", 512, 64, "12"), ("23", 256, 32, "22")]

_NC_CACHE = {}
DEBUG = False
# pointwise variant per (m_tile % len): A=ACT prelu+exp; B=DVE lrelu + ACT
# exp-half + DVE square-mask; G2=DVE s02 + GP max + ACT exp + GP mask;
# GM=A but mask on GP.  hcopy engine: a=ACT, v=DVE.
VARIANTS = ["GB", "A", "GB", "B", "GB", "GB", "A", "GB",
            "B", "GB", "GB", "GB", "A", "GB", "B", "B"]
HCOPY = {"A": "a", "B": "a", "G2": "a", "GM": "a", "GB": "a"}


def _build(repeat=1, no_collective=False):
    nc = bacc.Bacc("TRN2", target_bir_lowering=False, debug=False,
                   num_devices=NCORES)

    # ---------------- DRAM I/O ----------------
    xT0_d = nc.dram_tensor("xT0", [FIN, N], FP16, kind="ExternalInput")
    xo0_d = nc.dram_tensor("xo0", [FIN, R], FP16, kind="ExternalInput")
    aT_d = nc.dram_tensor("aT", [N, R], FP16, kind="ExternalInput")
    es1_d = nc.dram_tensor("es1", [N], FP16, kind="ExternalInput")
    es2_d = nc.dram_tensor("es2", [N], FP16, kind="ExternalInput")
    wc_d, ws_d, b_d = {}, {}, {}
    for (nm, F, C) in LAYERS:
        HC = H * C
        # Wcat = [W.reshape(F, HC) | Wt]  (Wt = einsum(W, at))
        wc_d[nm] = nc.dram_tensor(f"Wc{nm}", [F, HC + H], FP16,
                                  kind="ExternalInput")
        ws_d[nm] = nc.dram_tensor(f"Ws{nm}", [F, H], FP16, kind="ExternalInput")
        b_d[nm] = nc.dram_tensor(f"b{nm}", [HC], FP32, kind="ExternalInput")
    pool_d = nc.dram_tensor("pool", [768], FP32, kind="ExternalOutput")

    dbg_d = {}
    if DEBUG:
        for li, (nm, F, C) in enumerate(LAYERS):
            OC = (H * C) // 128
            dbg_d[nm] = nc.dram_tensor(f"dbg{nm}", [128, OC * R], FP16,
                                       kind="ExternalOutput")

    # internal DRAM: es scratch per layer + allgather buffers per transition
    es_scr, ag_in, ag_out = {}, {}, {}
    for rep in range(repeat):
        for li, (nm, F, C) in enumerate(LAYERS):
            key = (rep, nm)
            es_scr[key] = nc.dram_tensor(f"esscr{rep}_{nm}", [N], FP16,
                                         kind="Internal")
            if li % 3 != 2:
                # carries h_own = xn @ W_next (+et cols) for the NEXT layer:
                # [own 256 rows, HC_next+H]; gathered -> h for all 2048 nodes
                nF, nC = LAYERS[[x[0] for x in LAYERS].index(nm) + 1][1:3]
                nW2 = H * nC + H
                ag_in[key] = nc.dram_tensor(f"agin{rep}_{nm}", [R, nW2], FP16,
                                            kind="Internal")
                ag_out[key] = nc.dram_tensor(
                    f"agout{rep}_{nm}", [NCORES * R, nW2], FP16,
                    kind="Internal", addr_space="Shared")

    with tile.TileContext(nc) as tc:
        with ExitStack() as ctx:
            pl = lambda **kw: ctx.enter_context(tc.tile_pool(**kw))  # noqa: E731
            constp = pl(name="const", bufs=1)
            wmp = pl(name="wm", bufs=2)
            wsmp = pl(name="wsm", bufs=2)
            xtb1p = pl(name="xtb1", bufs=1)
            xtb2p = pl(name="xtb2", bufs=1)
            hp = pl(name="hsb", bufs=6)
            esbp = pl(name="esb", bufs=1)
            essbp = pl(name="essb", bufs=2)
            etp = pl(name="etsb", bufs=4)
            sp = pl(name="s", bufs=4)
            up = pl(name="u", bufs=4)
            Pp = pl(name="P", bufs=4)
            pp = pl(name="p", bufs=6)
            xnp = pl(name="xn", bufs=2)
            dvp = pl(name="dv", bufs=8)
            dinvp = pl(name="dinv", bufs=2)
            dinvbp = pl(name="dinvb", bufs=2)
            poutp = pl(name="pout", bufs=1)
            php = pl(name="ph", bufs=2, space="PSUM")
            pep = pl(name="pe", bufs=1, space="PSUM")
            paggp = pl(name="pagg", bufs=4, space="PSUM")
            pDp = pl(name="pD", bufs=1, space="PSUM")

            # -------- prologue: resident inputs --------
            aT_sb = constp.tile([128, MT * R], FP16)          # [m | (mt, n)]
            aT_v = aT_d[:].rearrange("(t p) n -> p t n", p=128)
            _eng = [nc.gpsimd, nc.scalar, nc.gpsimd, nc.scalar]
            for j in range(4):
                _eng[j].dma_start(
                    aT_sb[:].rearrange("p (t n) -> p t n", t=MT)
                    [:, 4 * j:4 * j + 4, :],
                    aT_v[:, 4 * j:4 * j + 4, :])
            xT0_sb = constp.tile([128, N], FP16)
            nc.sync.dma_start(xT0_sb[:], xT0_d[:])
            xo0_sb = constp.tile([128, R], FP16)
            nc.sync.dma_start(xo0_sb[:], xo0_d[:])
            ones_sb = constp.tile([128, 1], FP16)
            nc.gpsimd.memset(ones_sb[:], 1.0)

            for rep in range(repeat):
                xn_prev = [None, None]   # per block: own transposed activation
                for li, (nm, F, C, prev) in enumerate(SCHED):
                    HC = H * C
                    FC = F // 128        # input chunks (of fan-in)
                    OC = HC // 128       # output chunks (of hc rows)
                    W2 = HC + H          # wcat width
                    blk = 0 if nm[0] == "1" else 1
                    lyr = int(nm[1]) - 1
                    fuse_et = (HC + H) <= 512   # block2: et inside h-matmul

                    # -------- weights (wc only for layer-0 h matmuls) -----
                    if lyr == 0:
                        wc_sb = wmp.tile([128, FC * W2], FP16, tag="wm")
                        nc.gpsimd.dma_start(
                            wc_sb[:].rearrange("p (c d) -> p c d", c=FC),
                            wc_d[nm][:].rearrange("(c p) d -> p c d", p=128))
                    ws_sb = wsmp.tile([128, FC * H], FP16, tag="ws")
                    nc.gpsimd.dma_start(
                        ws_sb[:].rearrange("p (c d) -> p c d", c=FC),
                        ws_d[nm][:].rearrange("(c p) d -> p c d", p=128))
                    b_sb = wsmp.tile([128, OC], FP32, tag="b")
                    nc.gpsimd.dma_start(
                        b_sb[:], b_d[nm][:].rearrange("(c p) -> p c", p=128))

                    # -------- h source --------
                    # lyr==0: compute h from resident xT0 per m-tile (below).
                    # lyr>0: h (+et cols) was computed by the previous layer's
                    # producer and AllGather'd as [2048 nodes, W2]; load it
                    # directly per 2-m-tile chunk -- no recompute, no copies.
                    if lyr == 0:
                        xT_sb = xT0_sb
                        xo_ap = xo0_sb
                    else:
                        xo_ap = xn_prev[blk]
                        gsrc = ag_out[(rep, prev)]
                        hg = []
                        for g in range(MT // 2):
                            h2 = hp.tile([128, 2 * W2], FP16, tag="h")
                            nc.sync.dma_start(
                                h2[:].rearrange("p (t d) -> p t d", t=2),
                                gsrc[:].rearrange("(t p) d -> p t d", p=128)
                                [:, 2 * g:2 * g + 2, :])
                            hg.append(h2)

                    # -------- es chain --------
                    if lyr == 0:
                        es_src = es1_d if blk == 0 else es2_d
                    else:
                        es_src = es_scr[(rep, nm)]
                        pes = pep.tile([8, R], FP32, tag="pe")
                        for fc in range(FC):
                            nc.tensor.matmul(
                                pes[:], ws_sb[:, fc * H:(fc + 1) * H],
                                xo_ap[:, fc * R:(fc + 1) * R],
                                start=(fc == 0), stop=(fc == FC - 1))
                        es_sb = essbp.tile([8, R], FP16, tag="es")
                        nc.scalar.copy(es_sb[:], pes[:])
                        nc.gpsimd.dma_start(
                            es_src[:].rearrange("(h n) -> h n", h=8), es_sb[:])
                    esb = esbp.tile([128, N], FP16, tag="esb")
                    for j in range(2):
                        nc.sync.dma_start(
                            esb[:, j * 1024:(j + 1) * 1024],
                            es_src[j * 1024:(j + 1) * 1024][None, :]
                            .to_broadcast((128, 1024)))

                    # -------- aggregation psum (live across m loop) --------
                    agg_q = []
                    n_agg = 4 if C == 64 else 2
                    for _qi in range(n_agg):
                        agg_t = paggp.tile([128, 512], FP32, tag="agg")
                        agg_q.append(agg_t)
                    pD = pDp.tile([128, 512], FP32, tag="pD")

                    for i in range(MT):
                        # h (+ et fused for block2) for m-tile i
                        var = VARIANTS[i % len(VARIANTS)]
                        et_t = etp.tile([128, H], FP32, tag="et")
                        if lyr == 0:
                            ph = php.tile([128, W2 if fuse_et else HC], FP32,
                                          tag="ph")
                            rw = W2 if fuse_et else HC
                            for fc in range(FC):
                                lhs = xT_sb[:, fc * N + i * 128:
                                            fc * N + (i + 1) * 128]
                                nc.tensor.matmul(
                                    ph[:], lhs,
                                    wc_sb[:, fc * W2: fc * W2 + rw],
                                    start=(fc == 0), stop=(fc == FC - 1))
                            if not fuse_et:
                                pet = pep.tile([128, H], FP32, tag="pe")
                                for fc in range(FC):
                                    lhs = xT_sb[:, fc * N + i * 128:
                                                fc * N + (i + 1) * 128]
                                    nc.tensor.matmul(
                                        pet[:], lhs,
                                        wc_sb[:, fc * W2 + HC: (fc + 1) * W2],
                                        start=(fc == 0), stop=(fc == FC - 1))
                            h_sb = hp.tile([128, HC], FP16, tag="h")
                            nc.scalar.copy(h_sb[:], ph[:, 0:HC])
                            nc.scalar.copy(et_t[:], ph[:, HC:W2] if fuse_et
                                           else pet[:])
                            h_ap = h_sb[:]
                        else:
                            co = (i % 2) * W2
                            h_ap = hg[i // 2][:, co:co + HC]
                            nc.scalar.copy(
                                et_t[:], hg[i // 2][:, co + HC:co + W2])
                        et_ap = et_t

                        # pointwise: p = aT * exp(lrelu_0.2(es + et))
                        # variant A (ACT-heavy): Prelu + Exp on ACT, mask DVE
                        # variant B (DVE-heavy): lrelu = max(s, .2s) on DVE,
                        #   exp(u/2) on ACT, then p = (A2*aT)*A2 on DVE
                        s_t = sp.tile([128, N], FP16, tag="s")
                        for h in range(H):
                            nc.vector.tensor_scalar(
                                s_t[:, h * R:(h + 1) * R],
                                esb[:, h * R:(h + 1) * R],
                                et_ap[:, h: h + 1], None,
                                mybir.AluOpType.add)
                        aT_ap = (aT_sb[:, i * R:(i + 1) * R][:, None, :]
                                 .to_broadcast((128, H, R)))
                        p_t = pp.tile([128, N], FP16, tag="p")
                        if var in ("A", "GM"):
                            u_t = up.tile([128, N], FP16, tag="u")
                            nc.scalar.activation(
                                u_t[:], s_t[:],
                                mybir.ActivationFunctionType.Prelu, alpha=0.2)
                            P_t = Pp.tile([128, N], FP16, tag="P")
                            nc.scalar.activation(
                                P_t[:], u_t[:],
                                mybir.ActivationFunctionType.Exp)
                            eng = nc.vector if var == "A" else nc.gpsimd
                            eng.tensor_tensor(
                                p_t[:].rearrange("p (h n) -> p h n", h=H),
                                P_t[:].rearrange("p (h n) -> p h n", h=H),
                                aT_ap, mybir.AluOpType.mult)
                        elif var == "GB":
                            # DVE lrelu, ACT exp, GP mask
                            s2_t = up.tile([128, N], FP16, tag="u")
                            nc.vector.tensor_scalar(
                                s2_t[:], s_t[:], 0.2, None,
                                mybir.AluOpType.mult)
                            u2_t = up.tile([128, N], FP16, tag="u")
                            nc.vector.tensor_tensor(
                                u2_t[:], s_t[:], s2_t[:],
                                mybir.AluOpType.max)
                            P_t = Pp.tile([128, N], FP16, tag="P")
                            nc.scalar.activation(
                                P_t[:], u2_t[:],
                                mybir.ActivationFunctionType.Exp)
                            nc.gpsimd.tensor_tensor(
                                p_t[:].rearrange("p (h n) -> p h n", h=H),
                                P_t[:].rearrange("p (h n) -> p h n", h=H),
                                aT_ap, mybir.AluOpType.mult)
                        elif var == "B":
                            # lrelu on DVE (max(s, .2s)), exp on ACT, mask DVE
                            s2_t = up.tile([128, N], FP16, tag="u")
                            nc.vector.tensor_scalar(
                                s2_t[:], s_t[:], 0.2, None,
                                mybir.AluOpType.mult)
                            u2_t = up.tile([128, N], FP16, tag="u")
                            nc.vector.tensor_tensor(
                                u2_t[:], s_t[:], s2_t[:],
                                mybir.AluOpType.max)
                            P_t = Pp.tile([128, N], FP16, tag="P")
                            nc.scalar.activation(
                                P_t[:], u2_t[:],
                                mybir.ActivationFunctionType.Exp)
                            nc.vector.tensor_tensor(
                                p_t[:].rearrange("p (h n) -> p h n", h=H),
                                P_t[:].rearrange("p (h n) -> p h n", h=H),
                                aT_ap, mybir.AluOpType.mult)
                        else:  # G2: DVE s02, GP max, ACT exp, GP mask
                            s2_t = up.tile([128, N], FP16, tag="u")
                            nc.vector.tensor_scalar(
                                s2_t[:], s_t[:], 0.2, None,
                                mybir.AluOpType.mult)
                            nc.gpsimd.tensor_tensor(
                                s_t[:], s_t[:], s2_t[:],
                                mybir.AluOpType.max)
                            P_t = Pp.tile([128, N], FP16, tag="P")
                            nc.scalar.activation(
                                P_t[:], s_t[:],
                                mybir.ActivationFunctionType.Exp)
                            nc.gpsimd.tensor_tensor(
                                p_t[:].rearrange("p (h n) -> p h n", h=H),
                                P_t[:].rearrange("p (h n) -> p h n", h=H),
                                aT_ap, mybir.AluOpType.mult)

                        # aggregation: 2 heads per matmul ([128, 512] rhs).
                        # One accumulation group per (partition-range, bank).
                        for j in range(4):
                            lhsT = h_ap[:, j * 2 * C:(j + 1) * 2 * C]
                            rhs = p_t[:, j * 512:(j + 1) * 512]
                            if C == 64:
                                nc.tensor.matmul(
                                    agg_q[j][:, :], lhsT, rhs,
                                    start=(i == 0), stop=(i == MT - 1),
                                    tile_position=(0, 0))
                            else:
                                pb = (j % 2) * 64
                                nc.tensor.matmul(
                                    agg_q[j // 2][pb:pb + 64, :], lhsT, rhs,
                                    start=(i == 0), stop=(i == MT - 1),
                                    tile_position=(0, pb),
                                    skip_group_check=(pb > 0))
                        for j in range(4):
                            nc.tensor.matmul(
                                pD[32 * j:32 * j + 1, :],
                                ones_sb[:],
                                p_t[:, j * 512:(j + 1) * 512],
                                start=(i == 0), stop=(i == MT - 1),
                                tile_position=(0, 32 * j),
                                skip_group_check=(j > 0))

                    # -------- finalize: alpha-normalize + bias + relu ------
                    # pipelined per head-pair: recip chunk (DVE) -> broadcast
                    # chunk (GP, from partition 0) -> normalize+relu, so the
                    # ag_in payload is ready ~8us earlier than monolithic
                    # stages and the AllGather starts sooner
                    dinv = dinvp.tile([1, N], FP32, tag="dinv")
                    dinvb = dinvbp.tile([128, N], FP32, tag="dinvb")
                    xn = xnp.tile([128, OC * R], FP16, tag=f"xn{blk}")
                    hpc = 128 // C  # heads per 128-row chunk
                    for j in range(4):
                        sl = slice(j * 512, (j + 1) * 512)
                        nc.vector.reciprocal(dinv[0:1, sl],
                                             pD[32 * j:32 * j + 1, :])
                        nc.gpsimd.partition_broadcast(dinvb[:, sl],
                                                      dinv[0:1, sl])
                        for h in (2 * j, 2 * j + 1):
                            t, k = divmod(h, hpc)
                            pb = k * C
                            fo = (k % 2) * 256
                            # b1: chunk t = pair tile t; b2: tile t, see map
                            src = agg_q[t][pb:pb + C, fo:fo + R]
                            dv = dvp.tile([128, R], FP32, tag="dv")
                            nc.vector.tensor_tensor(
                                dv[pb:pb + C, :], src,
                                dinvb[pb:pb + C, h * R:(h + 1) * R],
                                mybir.AluOpType.mult)
                            nc.scalar.activation(
                                xn[pb:pb + C, t * R:(t + 1) * R],
                                dv[pb:pb + C, :],
                                mybir.ActivationFunctionType.Relu,
                                bias=b_sb[pb:pb + C, t:t + 1])

                    if DEBUG:
                        nc.sync.dma_start(dbg_d[nm][:, 0:OC * R], xn[:])
                    if lyr == 2:
                        # global pool: partial sum over own 256 rows
                        po = poutp.tile([128, OC], FP32, tag=f"po{blk}")
                        for t in range(OC):
                            nc.vector.tensor_reduce(
                                po[:, t:t + 1], xn[:, t * R:(t + 1) * R],
                                axis=mybir.AxisListType.X,
                                op=mybir.AluOpType.add)
                        off = 0 if blk == 0 else 512
                        nc.sync.dma_start(
                            pool_d[off:off + HC].rearrange("(c p) -> p c",
                                                           p=128),
                            po[:])
                    else:
                        xn_prev[blk] = xn
                        # producer: next layer's h_own = xn @ Wc_next for the
                        # own 256 rows only; the AllGather then distributes
                        # finished h (+et cols), so no core recomputes h for
                        # all 2048 nodes.  Output dims (HC, W2) match this
                        # layer's (C constant within a block).
                        nmx = nm[0] + str(int(nm[1]) + 1)
                        wcn = wmp.tile([128, OC * W2], FP16, tag="wm")
                        nc.gpsimd.dma_start(
                            wcn[:].rearrange("p (c d) -> p c d", c=OC),
                            wc_d[nmx][:].rearrange("(c p) d -> p c d", p=128))
                        for mh in range(2):
                            phh = php.tile([128, HC], FP32, tag="ph")
                            pee = pep.tile([128, H], FP32, tag="pe")
                            for fc in range(OC):
                                lhs = xn[:, fc * R + mh * 128:
                                         fc * R + mh * 128 + 128]
                                nc.tensor.matmul(
                                    phh[:], lhs,
                                    wcn[:, fc * W2:fc * W2 + HC],
                                    start=(fc == 0), stop=(fc == OC - 1))
                                nc.tensor.matmul(
                                    pee[:], lhs,
                                    wcn[:, fc * W2 + HC:(fc + 1) * W2],
                                    start=(fc == 0), stop=(fc == OC - 1))
                            h8 = hp.tile([128, W2], FP16, tag="h")
                            nc.scalar.copy(h8[:, 0:HC], phh[:])
                            nc.scalar.copy(h8[:, HC:W2], pee[:])
                            nc.sync.dma_start(
                                ag_in[(rep, nm)][mh * 128:(mh + 1) * 128, :],
                                h8[:])
                        if no_collective:
                            for r in range(NCORES):
                                nc.sync.dma_start(
                                    ag_out[(rep, nm)][r * HC:(r + 1) * HC, :],
                                    ag_in[(rep, nm)][:])
                        else:
                            nc.gpsimd.collective_compute(
                                "AllGather", mybir.AluOpType.bypass,
                                replica_groups=[list(range(NCORES))],
                                ins=[ag_in[(rep, nm)][:].opt()],
                                outs=[ag_out[(rep, nm)][:].opt()])

    nc.compile()
    return nc


def _get_nc():
    if "nc" not in _NC_CACHE:
        _NC_CACHE["nc"] = _build()
    return _NC_CACHE["nc"]


def _prep_inputs(inputs):
    f16 = np.float16
    x = np.asarray(inputs["x"], np.float32)
    a = np.asarray(inputs["a"], np.float32)
    base = {}
    base["xT0"] = np.ascontiguousarray(x.T).astype(f16)
    for (nm, F, C) in LAYERS:
        W = np.asarray(inputs["W" + nm], np.float32)   # [F, H, C]
        at = np.asarray(inputs["at" + nm], np.float32)  # [H, C]
        as_ = np.asarray(inputs["as" + nm], np.float32)
        wt = np.einsum("fhc,hc->fh", W, at)
        wcat = np.concatenate([W.reshape(F, H * C), wt], axis=1)
        base["Wc" + nm] = np.ascontiguousarray(wcat).astype(f16)
        base["Ws" + nm] = np.ascontiguousarray(
            np.einsum("fhc,hc->fh", W, as_)).astype(f16)
        base["b" + nm] = np.asarray(inputs["b" + nm], np.float32)
    maps = []
    xb = x.astype(np.float16).astype(np.float32)  # match device fp16
    for c in range(NCORES):
        m = dict(base)
        m["aT"] = np.ascontiguousarray(a[c * R:(c + 1) * R, :].T).astype(f16)
        m["xo0"] = np.ascontiguousarray(x[c * R:(c + 1) * R, :].T).astype(f16)
        xo = xb[c * R:(c + 1) * R, :]
        for blk, nm in ((0, "11"), (1, "21")):
            W = np.asarray(inputs["W" + nm], np.float32)
            as_ = np.asarray(inputs["as" + nm], np.float32)
            ws = np.einsum("fhc,hc->fh", W, as_)
            ws = ws.astype(np.float16).astype(np.float32)
            es = xo @ ws                       # [R, H]
            m["es1" if blk == 0 else "es2"] = np.ascontiguousarray(
                es.T.reshape(-1)).astype(np.float16)
        maps.append(m)
    return maps


def kernel(**inputs):
    nc = _get_nc()
    maps = _prep_inputs(inputs)
    res = run_bass_kernel_spmd(nc, maps, core_ids=list(range(NCORES)))
    out = np.zeros(768, np.float64)
    for c in range(NCORES):
        out += res.results[c]["pool"].astype(np.float64)
    return out.astype(np.float32)


if __name__ == "__main__":
    rng = np.random.default_rng(0)
    ins = {"x": rng.standard_normal((N, FIN)).astype(np.float32),
           "a": (rng.random((N, N)) < 0.01).astype(np.float32)}
    for (nm, F, C) in LAYERS:
        ins["W" + nm] = (rng.standard_normal((F, H, C)) / np.sqrt(F)).astype(np.float32)
        ins["as" + nm] = (rng.standard_normal((H, C)) * 0.1).astype(np.float32)
        ins["at" + nm] = (rng.standard_normal((H, C)) * 0.1).astype(np.float32)
        ins["b" + nm] = np.zeros(H * C, np.float32)
    out = kernel(**ins)
    print("kernel out[:8] =", out[:8])



# revision 48
# speedup vs baseline: 1.2568x; 1.0078x over previous
"""GAT (2-block, 3-layer) Trainium2 Bass kernel, 8-core SPMD.

Sharding: target-node rows (n) split across 8 cores (256 rows each).
Per layer, each core computes h = x @ W for ALL source nodes (needs the
full activation, obtained via AllGather), then row-local masked softmax
attention + aggregation for its 256 target rows.  The aggregation matmul
produces the TRANSPOSED activation [hc, n_own] which is exactly the
layout needed as lhsT for the next layer -- no transposes anywhere.
Final pooled vectors are partial-summed per core and reduced on host.

Self-contained: hardcodes all shapes; only needs /opt/trn_rl_repo.
"""
import sys
from contextlib import ExitStack

import numpy as np

sys.path.insert(0, "/opt/trn_rl_repo")

import concourse.bass as bass  # noqa: E402
import concourse.bacc as bacc  # noqa: E402
import concourse.tile as tile  # noqa: E402
from concourse import mybir  # noqa: E402
from concourse.bass_utils import run_bass_kernel_spmd  # noqa: E402

N = 2048
FIN = 128
H = 8
NCORES = 8
R = N // NCORES          # 256 target rows per core
MT = N // 128            # 16 source m-tiles
FP32 = mybir.dt.float32
FP16 = mybir.dt.float16

# (name, fan_in, C) ; blocks: 0 = layers *1x (C=64), 1 = *2x (C=32)
LAYERS = [("11", 128, 64), ("12", 512, 64), ("13", 512, 64),
          ("21", 128, 32), ("22", 256, 32), ("23", 256, 32)]
# emission order interleaves the two independent blocks so one block's
# compute hides the other's AllGather transition
SCHED = [("11", 128, 64, None), ("21", 128, 32, None),
         ("12", 512, 64, "11"), ("22", 256, 32, "21"),
         ("13", 512, 64, "12"), ("23", 256, 32, "22")]

_NC_CACHE = {}
DEBUG = False
# pointwise variant per (m_tile % len): A=ACT prelu+exp; B=DVE lrelu + ACT
# exp-half + DVE square-mask; G2=DVE s02 + GP max + ACT exp + GP mask;
# GM=A but mask on GP.  hcopy engine: a=ACT, v=DVE.
VARIANTS = ["GB", "A", "GB", "B", "GB", "GB", "A", "GB",
            "B", "GB", "GB", "GB", "A", "GB", "B", "B"]
HCOPY = {"A": "a", "B": "a", "G2": "a", "GM": "a", "GB": "a"}


def _build(repeat=1, no_collective=False):
    nc = bacc.Bacc("TRN2", target_bir_lowering=False, debug=False,
                   num_devices=NCORES)

    # ---------------- DRAM I/O ----------------
    xT0_d = nc.dram_tensor("xT0", [FIN, N], FP16, kind="ExternalInput")
    xo0_d = nc.dram_tensor("xo0", [FIN, R], FP16, kind="ExternalInput")
    aT_d = nc.dram_tensor("aT", [N, R], FP16, kind="ExternalInput")
    es1_d = nc.dram_tensor("es1", [N], FP16, kind="ExternalInput")
    es2_d = nc.dram_tensor("es2", [N], FP16, kind="ExternalInput")
    wc_d, ws_d, b_d = {}, {}, {}
    for (nm, F, C) in LAYERS:
        HC = H * C
        # Wcat = [W.reshape(F, HC) | Wt]  (Wt = einsum(W, at))
        wc_d[nm] = nc.dram_tensor(f"Wc{nm}", [F, HC + H], FP16,
                                  kind="ExternalInput")
        ws_d[nm] = nc.dram_tensor(f"Ws{nm}", [F, H], FP16, kind="ExternalInput")
        b_d[nm] = nc.dram_tensor(f"b{nm}", [HC], FP32, kind="ExternalInput")
    pool_d = nc.dram_tensor("pool", [768], FP32, kind="ExternalOutput")

    dbg_d = {}
    if DEBUG:
        for li, (nm, F, C) in enumerate(LAYERS):
            OC = (H * C) // 128
            dbg_d[nm] = nc.dram_tensor(f"dbg{nm}", [128, OC * R], FP16,
                                       kind="ExternalOutput")

    # internal DRAM: es scratch per layer + allgather buffers per transition
    es_scr, ag_in, ag_out = {}, {}, {}
    for rep in range(repeat):
        for li, (nm, F, C) in enumerate(LAYERS):
            key = (rep, nm)
            es_scr[key] = nc.dram_tensor(f"esscr{rep}_{nm}", [N], FP16,
                                         kind="Internal")
            if li % 3 != 2:
                # carries h_own = xn @ W_next (+et cols) for the NEXT layer:
                # [own 256 rows, HC_next+H]; gathered -> h for all 2048 nodes
                nF, nC = LAYERS[[x[0] for x in LAYERS].index(nm) + 1][1:3]
                nW2 = H * nC + H
                ag_in[key] = nc.dram_tensor(f"agin{rep}_{nm}", [R, nW2], FP16,
                                            kind="Internal")
                ag_out[key] = nc.dram_tensor(
                    f"agout{rep}_{nm}", [NCORES * R, nW2], FP16,
                    kind="Internal", addr_space="Shared")

    with tile.TileContext(nc) as tc:
        with ExitStack() as ctx:
            pl = lambda **kw: ctx.enter_context(tc.tile_pool(**kw))  # noqa: E731
            constp = pl(name="const", bufs=1)
            wmp = pl(name="wm", bufs=2)
            wsmp = pl(name="wsm", bufs=2)
            xtb1p = pl(name="xtb1", bufs=1)
            xtb2p = pl(name="xtb2", bufs=1)
            hp = pl(name="hsb", bufs=6)
            esbp = pl(name="esb", bufs=1)
            essbp = pl(name="essb", bufs=2)
            etp = pl(name="etsb", bufs=4)
            sp = pl(name="s", bufs=4)
            up = pl(name="u", bufs=4)
            Pp = pl(name="P", bufs=4)
            pp = pl(name="p", bufs=6)
            xnp = pl(name="xn", bufs=2)
            dvp = pl(name="dv", bufs=8)
            dinvp = pl(name="dinv", bufs=2)
            dinvbp = pl(name="dinvb", bufs=2)
            poutp = pl(name="pout", bufs=1)
            php = pl(name="ph", bufs=2, space="PSUM")
            pep = pl(name="pe", bufs=1, space="PSUM")
            paggp = pl(name="pagg", bufs=4, space="PSUM")
            pDp = pl(name="pD", bufs=1, space="PSUM")

            # -------- prologue: resident inputs --------
            aT_sb = constp.tile([128, MT * R], FP16)          # [m | (mt, n)]
            aT_v = aT_d[:].rearrange("(t p) n -> p t n", p=128)
            _eng = [nc.gpsimd, nc.scalar, nc.gpsimd, nc.scalar]
            for j in range(4):
                _eng[j].dma_start(
                    aT_sb[:].rearrange("p (t n) -> p t n", t=MT)
                    [:, 4 * j:4 * j + 4, :],
                    aT_v[:, 4 * j:4 * j + 4, :])
            xT0_sb = constp.tile([128, N], FP16)
            nc.sync.dma_start(xT0_sb[:], xT0_d[:])
            xo0_sb = constp.tile([128, R], FP16)
            nc.sync.dma_start(xo0_sb[:], xo0_d[:])
            ones_sb = constp.tile([128, 1], FP16)
            nc.gpsimd.memset(ones_sb[:], 1.0)

            for rep in range(repeat):
                xn_prev = [None, None]   # per block: own transposed activation
                for li, (nm, F, C, prev) in enumerate(SCHED):
                    HC = H * C
                    FC = F // 128        # input chunks (of fan-in)
                    OC = HC // 128       # output chunks (of hc rows)
                    W2 = HC + H          # wcat width
                    blk = 0 if nm[0] == "1" else 1
                    lyr = int(nm[1]) - 1
                    fuse_et = (HC + H) <= 512   # block2: et inside h-matmul

                    # -------- weights (wc only for layer-0 h matmuls) -----
                    if lyr == 0:
                        wc_sb = wmp.tile([128, FC * W2], FP16, tag="wm")
                        nc.gpsimd.dma_start(
                            wc_sb[:].rearrange("p (c d) -> p c d", c=FC),
                            wc_d[nm][:].rearrange("(c p) d -> p c d", p=128))
                    ws_sb = wsmp.tile([128, FC * H], FP16, tag="ws")
                    nc.gpsimd.dma_start(
                        ws_sb[:].rearrange("p (c d) -> p c d", c=FC),
                        ws_d[nm][:].rearrange("(c p) d -> p c d", p=128))
                    b_sb = wsmp.tile([128, OC], FP32, tag="b")
                    nc.gpsimd.dma_start(
                        b_sb[:], b_d[nm][:].rearrange("(c p) -> p c", p=128))

                    # -------- h source --------
                    # lyr==0: compute h from resident xT0 per m-tile (below).
                    # lyr>0: h (+et cols) was computed by the previous layer's
                    # producer and AllGather'd as [2048 nodes, W2]; load it
                    # directly per 2-m-tile chunk -- no recompute, no copies.
                    if lyr == 0:
                        xT_sb = xT0_sb
                        xo_ap = xo0_sb
                    else:
                        xo_ap = xn_prev[blk]
                        gsrc = ag_out[(rep, prev)]
                        hg = []
                        for g in range(MT // 2):
                            h2 = hp.tile([128, 2 * W2], FP16, tag="h")
                            nc.sync.dma_start(
                                h2[:].rearrange("p (t d) -> p t d", t=2),
                                gsrc[:].rearrange("(t p) d -> p t d", p=128)
                                [:, 2 * g:2 * g + 2, :])
                            hg.append(h2)

                    # -------- es chain --------
                    if lyr == 0:
                        es_src = es1_d if blk == 0 else es2_d
                    else:
                        es_src = es_scr[(rep, nm)]
                        pes = pep.tile([8, R], FP32, tag="pe")
                        for fc in range(FC):
                            nc.tensor.matmul(
                                pes[:], ws_sb[:, fc * H:(fc + 1) * H],
                                xo_ap[:, fc * R:(fc + 1) * R],
                                start=(fc == 0), stop=(fc == FC - 1))
                        es_sb = essbp.tile([8, R], FP16, tag="es")
                        nc.scalar.copy(es_sb[:], pes[:])
                        nc.gpsimd.dma_start(
                            es_src[:].rearrange("(h n) -> h n", h=8), es_sb[:])
                    esb = esbp.tile([128, N], FP16, tag="esb")
                    for j in range(2):
                        nc.sync.dma_start(
                            esb[:, j * 1024:(j + 1) * 1024],
                            es_src[j * 1024:(j + 1) * 1024][None, :]
                            .to_broadcast((128, 1024)))

                    # -------- aggregation psum (live across m loop) --------
                    agg_q = []
                    n_agg = 4 if C == 64 else 2
                    for _qi in range(n_agg):
                        agg_t = paggp.tile([128, 512], FP32, tag="agg")
                        agg_q.append(agg_t)
                    pD = pDp.tile([128, 512], FP32, tag="pD")

                    for i in range(MT):
                        # h (+ et fused for block2) for m-tile i
                        var = VARIANTS[i % len(VARIANTS)]
                        et_t = etp.tile([128, H], FP32, tag="et")
                        if lyr == 0:
                            ph = php.tile([128, W2 if fuse_et else HC], FP32,
                                          tag="ph")
                            rw = W2 if fuse_et else HC
                            for fc in range(FC):
                                lhs = xT_sb[:, fc * N + i * 128:
                                            fc * N + (i + 1) * 128]
                                nc.tensor.matmul(
                                    ph[:], lhs,
                                    wc_sb[:, fc * W2: fc * W2 + rw],
                                    start=(fc == 0), stop=(fc == FC - 1))
                            if not fuse_et:
                                pet = pep.tile([128, H], FP32, tag="pe")
                                for fc in range(FC):
                                    lhs = xT_sb[:, fc * N + i * 128:
                                                fc * N + (i + 1) * 128]
                                    nc.tensor.matmul(
                                        pet[:], lhs,
                                        wc_sb[:, fc * W2 + HC: (fc + 1) * W2],
                                        start=(fc == 0), stop=(fc == FC - 1))
                            h_sb = hp.tile([128, HC], FP16, tag="h")
                            nc.scalar.copy(h_sb[:], ph[:, 0:HC])
                            nc.scalar.copy(et_t[:], ph[:, HC:W2] if fuse_et
                                           else pet[:])
                            h_ap = h_sb[:]
                        else:
                            co = (i % 2) * W2
                            h_ap = hg[i // 2][:, co:co + HC]
                            nc.scalar.copy(
                                et_t[:], hg[i // 2][:, co + HC:co + W2])
                        et_ap = et_t

                        # pointwise: p = aT * exp(lrelu_0.2(es + et))
                        # variant A (ACT-heavy): Prelu + Exp on ACT, mask DVE
                        # variant B (DVE-heavy): lrelu = max(s, .2s) on DVE,
                        #   exp(u/2) on ACT, then p = (A2*aT)*A2 on DVE
                        if var not in ("A", "GM"):
                            s_t = sp.tile([128, N], FP16, tag="s")
                            for h in range(H):
                                nc.vector.tensor_scalar(
                                    s_t[:, h * R:(h + 1) * R],
                                    esb[:, h * R:(h + 1) * R],
                                    et_ap[:, h: h + 1], None,
                                    mybir.AluOpType.add)
                        aT_ap = (aT_sb[:, i * R:(i + 1) * R][:, None, :]
                                 .to_broadcast((128, H, R)))
                        p_t = pp.tile([128, N], FP16, tag="p")
                        if var in ("A", "GM"):
                            # es+et add folded into the Prelu bias (per head)
                            # -- no DVE s-adds for these tiles
                            u_t = up.tile([128, N], FP16, tag="u")
                            for h in range(H):
                                nc.scalar.activation(
                                    u_t[:, h * R:(h + 1) * R],
                                    esb[:, h * R:(h + 1) * R],
                                    mybir.ActivationFunctionType.Prelu,
                                    bias=et_ap[:, h:h + 1], alpha=0.2)
                            P_t = Pp.tile([128, N], FP16, tag="P")
                            nc.scalar.activation(
                                P_t[:], u_t[:],
                                mybir.ActivationFunctionType.Exp)
                            eng = nc.vector if var == "A" else nc.gpsimd
                            eng.tensor_tensor(
                                p_t[:].rearrange("p (h n) -> p h n", h=H),
                                P_t[:].rearrange("p (h n) -> p h n", h=H),
                                aT_ap, mybir.AluOpType.mult)
                        elif var == "GB":
                            # DVE lrelu, ACT exp, GP mask
                            s2_t = up.tile([128, N], FP16, tag="u")
                            nc.vector.tensor_scalar(
                                s2_t[:], s_t[:], 0.2, None,
                                mybir.AluOpType.mult)
                            u2_t = up.tile([128, N], FP16, tag="u")
                            nc.vector.tensor_tensor(
                                u2_t[:], s_t[:], s2_t[:],
                                mybir.AluOpType.max)
                            P_t = Pp.tile([128, N], FP16, tag="P")
                            nc.scalar.activation(
                                P_t[:], u2_t[:],
                                mybir.ActivationFunctionType.Exp)
                            nc.gpsimd.tensor_tensor(
                                p_t[:].rearrange("p (h n) -> p h n", h=H),
                                P_t[:].rearrange("p (h n) -> p h n", h=H),
                                aT_ap, mybir.AluOpType.mult)
                        elif var == "B":
                            # lrelu on DVE (max(s, .2s)), exp on ACT, mask DVE
                            s2_t = up.tile([128, N], FP16, tag="u")
                            nc.vector.tensor_scalar(
                                s2_t[:], s_t[:], 0.2, None,
                                mybir.AluOpType.mult)
                            u2_t = up.tile([128, N], FP16, tag="u")
                            nc.vector.tensor_tensor(
                                u2_t[:], s_t[:], s2_t[:],
                                mybir.AluOpType.max)
                            P_t = Pp.tile([128, N], FP16, tag="P")
                            nc.scalar.activation(
                                P_t[:], u2_t[:],
                                mybir.ActivationFunctionType.Exp)
                            nc.vector.tensor_tensor(
                                p_t[:].rearrange("p (h n) -> p h n", h=H),
                                P_t[:].rearrange("p (h n) -> p h n", h=H),
                                aT_ap, mybir.AluOpType.mult)
                        else:  # G2: DVE s02, GP max, ACT exp, GP mask
                            s2_t = up.tile([128, N], FP16, tag="u")
                            nc.vector.tensor_scalar(
                                s2_t[:], s_t[:], 0.2, None,
                                mybir.AluOpType.mult)
                            nc.gpsimd.tensor_tensor(
                                s_t[:], s_t[:], s2_t[:],
                                mybir.AluOpType.max)
                            P_t = Pp.tile([128, N], FP16, tag="P")
                            nc.scalar.activation(
                                P_t[:], s_t[:],
                                mybir.ActivationFunctionType.Exp)
                            nc.gpsimd.tensor_tensor(
                                p_t[:].rearrange("p (h n) -> p h n", h=H),
                                P_t[:].rearrange("p (h n) -> p h n", h=H),
                                aT_ap, mybir.AluOpType.mult)

                        # aggregation: 2 heads per matmul ([128, 512] rhs).
                        # One accumulation group per (partition-range, bank).
                        for j in range(4):
                            lhsT = h_ap[:, j * 2 * C:(j + 1) * 2 * C]
                            rhs = p_t[:, j * 512:(j + 1) * 512]
                            if C == 64:
                                nc.tensor.matmul(
                                    agg_q[j][:, :], lhsT, rhs,
                                    start=(i == 0), stop=(i == MT - 1),
                                    tile_position=(0, 0))
                            else:
                                pb = (j % 2) * 64
                                nc.tensor.matmul(
                                    agg_q[j // 2][pb:pb + 64, :], lhsT, rhs,
                                    start=(i == 0), stop=(i == MT - 1),
                                    tile_position=(0, pb),
                                    skip_group_check=(pb > 0))
                        for j in range(4):
                            nc.tensor.matmul(
                                pD[32 * j:32 * j + 1, :],
                                ones_sb[:],
                                p_t[:, j * 512:(j + 1) * 512],
                                start=(i == 0), stop=(i == MT - 1),
                                tile_position=(0, 32 * j),
                                skip_group_check=(j > 0))

                    # -------- finalize: alpha-normalize + bias + relu ------
                    # pipelined per head-pair: recip chunk (DVE) -> broadcast
                    # chunk (GP, from partition 0) -> normalize+relu, so the
                    # ag_in payload is ready ~8us earlier than monolithic
                    # stages and the AllGather starts sooner
                    dinv = dinvp.tile([1, N], FP32, tag="dinv")
                    dinvb = dinvbp.tile([128, N], FP32, tag="dinvb")
                    xn = xnp.tile([128, OC * R], FP16, tag=f"xn{blk}")
                    hpc = 128 // C  # heads per 128-row chunk
                    for j in range(4):
                        sl = slice(j * 512, (j + 1) * 512)
                        nc.vector.reciprocal(dinv[0:1, sl],
                                             pD[32 * j:32 * j + 1, :])
                        nc.gpsimd.partition_broadcast(dinvb[:, sl],
                                                      dinv[0:1, sl])
                        for h in (2 * j, 2 * j + 1):
                            t, k = divmod(h, hpc)
                            pb = k * C
                            fo = (k % 2) * 256
                            # b1: chunk t = pair tile t; b2: tile t, see map
                            src = agg_q[t][pb:pb + C, fo:fo + R]
                            dv = dvp.tile([128, R], FP32, tag="dv")
                            nc.vector.tensor_tensor(
                                dv[pb:pb + C, :], src,
                                dinvb[pb:pb + C, h * R:(h + 1) * R],
                                mybir.AluOpType.mult)
                            nc.scalar.activation(
                                xn[pb:pb + C, t * R:(t + 1) * R],
                                dv[pb:pb + C, :],
                                mybir.ActivationFunctionType.Relu,
                                bias=b_sb[pb:pb + C, t:t + 1])

                    if DEBUG:
                        nc.sync.dma_start(dbg_d[nm][:, 0:OC * R], xn[:])
                    if lyr == 2:
                        # global pool: partial sum over own 256 rows
                        po = poutp.tile([128, OC], FP32, tag=f"po{blk}")
                        for t in range(OC):
                            nc.vector.tensor_reduce(
                                po[:, t:t + 1], xn[:, t * R:(t + 1) * R],
                                axis=mybir.AxisListType.X,
                                op=mybir.AluOpType.add)
                        off = 0 if blk == 0 else 512
                        nc.sync.dma_start(
                            pool_d[off:off + HC].rearrange("(c p) -> p c",
                                                           p=128),
                            po[:])
                    else:
                        xn_prev[blk] = xn
                        # producer: next layer's h_own = xn @ Wc_next for the
                        # own 256 rows only; the AllGather then distributes
                        # finished h (+et cols), so no core recomputes h for
                        # all 2048 nodes.  Output dims (HC, W2) match this
                        # layer's (C constant within a block).
                        nmx = nm[0] + str(int(nm[1]) + 1)
                        wcn = wmp.tile([128, OC * W2], FP16, tag="wm")
                        nc.gpsimd.dma_start(
                            wcn[:].rearrange("p (c d) -> p c d", c=OC),
                            wc_d[nmx][:].rearrange("(c p) d -> p c d", p=128))
                        for mh in range(2):
                            phh = php.tile([128, HC], FP32, tag="ph")
                            pee = pep.tile([128, H], FP32, tag="pe")
                            for fc in range(OC):
                                lhs = xn[:, fc * R + mh * 128:
                                         fc * R + mh * 128 + 128]
                                nc.tensor.matmul(
                                    phh[:], lhs,
                                    wcn[:, fc * W2:fc * W2 + HC],
                                    start=(fc == 0), stop=(fc == OC - 1))
                                nc.tensor.matmul(
                                    pee[:], lhs,
                                    wcn[:, fc * W2 + HC:(fc + 1) * W2],
                                    start=(fc == 0), stop=(fc == OC - 1))
                            h8 = hp.tile([128, W2], FP16, tag="h")
                            nc.scalar.copy(h8[:, 0:HC], phh[:])
                            nc.scalar.copy(h8[:, HC:W2], pee[:])
                            nc.sync.dma_start(
                                ag_in[(rep, nm)][mh * 128:(mh + 1) * 128, :],
                                h8[:])
                        if no_collective:
                            for r in range(NCORES):
                                nc.sync.dma_start(
                                    ag_out[(rep, nm)][r * HC:(r + 1) * HC, :],
                                    ag_in[(rep, nm)][:])
                        else:
                            nc.gpsimd.collective_compute(
                                "AllGather", mybir.AluOpType.bypass,
                                replica_groups=[list(range(NCORES))],
                                ins=[ag_in[(rep, nm)][:].opt()],
                                outs=[ag_out[(rep, nm)][:].opt()])

    nc.compile()
    return nc


def _get_nc():
    if "nc" not in _NC_CACHE:
        _NC_CACHE["nc"] = _build()
    return _NC_CACHE["nc"]


def _prep_inputs(inputs):
    f16 = np.float16
    x = np.asarray(inputs["x"], np.float32)
    a = np.asarray(inputs["a"], np.float32)
    base = {}
    base["xT0"] = np.ascontiguousarray(x.T).astype(f16)
    for (nm, F, C) in LAYERS:
        W = np.asarray(inputs["W" + nm], np.float32)   # [F, H, C]
        at = np.asarray(inputs["at" + nm], np.float32)  # [H, C]
        as_ = np.asarray(inputs["as" + nm], np.float32)
        wt = np.einsum("fhc,hc->fh", W, at)
        wcat = np.concatenate([W.reshape(F, H * C), wt], axis=1)
        base["Wc" + nm] = np.ascontiguousarray(wcat).astype(f16)
        base["Ws" + nm] = np.ascontiguousarray(
            np.einsum("fhc,hc->fh", W, as_)).astype(f16)
        base["b" + nm] = np.asarray(inputs["b" + nm], np.float32)
    maps = []
    xb = x.astype(np.float16).astype(np.float32)  # match device fp16
    for c in range(NCORES):
        m = dict(base)
        m["aT"] = np.ascontiguousarray(a[c * R:(c + 1) * R, :].T).astype(f16)
        m["xo0"] = np.ascontiguousarray(x[c * R:(c + 1) * R, :].T).astype(f16)
        xo = xb[c * R:(c + 1) * R, :]
        for blk, nm in ((0, "11"), (1, "21")):
            W = np.asarray(inputs["W" + nm], np.float32)
            as_ = np.asarray(inputs["as" + nm], np.float32)
            ws = np.einsum("fhc,hc->fh", W, as_)
            ws = ws.astype(np.float16).astype(np.float32)
            es = xo @ ws                       # [R, H]
            m["es1" if blk == 0 else "es2"] = np.ascontiguousarray(
                es.T.reshape(-1)).astype(np.float16)
        maps.append(m)
    return maps


def kernel(**inputs):
    nc = _get_nc()
    maps = _prep_inputs(inputs)
    res = run_bass_kernel_spmd(nc, maps, core_ids=list(range(NCORES)))
    out = np.zeros(768, np.float64)
    for c in range(NCORES):
        out += res.results[c]["pool"].astype(np.float64)
    return out.astype(np.float32)


if __name__ == "__main__":
    rng = np.random.default_rng(0)
    ins = {"x": rng.standard_normal((N, FIN)).astype(np.float32),
           "a": (rng.random((N, N)) < 0.01).astype(np.float32)}
    for (nm, F, C) in LAYERS:
        ins["W" + nm] = (rng.standard_normal((F, H, C)) / np.sqrt(F)).astype(np.float32)
        ins["as" + nm] = (rng.standard_normal((H, C)) * 0.1).astype(np.float32)
        ins["at" + nm] = (rng.standard_normal((H, C)) * 0.1).astype(np.float32)
        ins["b" + nm] = np.zeros(H * C, np.float32)
    out = kernel(**ins)
    print("kernel out[:8] =", out[:8])



# revision 57
# speedup vs baseline: 1.2764x; 1.0156x over previous
"""GAT (2-block, 3-layer) Trainium2 Bass kernel, 8-core SPMD.

Sharding: target-node rows (n) split across 8 cores (256 rows each).
Per layer, each core computes h = x @ W for ALL source nodes (needs the
full activation, obtained via AllGather), then row-local masked softmax
attention + aggregation for its 256 target rows.  The aggregation matmul
produces the TRANSPOSED activation [hc, n_own] which is exactly the
layout needed as lhsT for the next layer -- no transposes anywhere.
Final pooled vectors are partial-summed per core and reduced on host.

Self-contained: hardcodes all shapes; only needs /opt/trn_rl_repo.
"""
import sys
from contextlib import ExitStack

import numpy as np

sys.path.insert(0, "/opt/trn_rl_repo")

import concourse.bass as bass  # noqa: E402
import concourse.bacc as bacc  # noqa: E402
import concourse.tile as tile  # noqa: E402
from concourse import mybir  # noqa: E402
from concourse.bass_utils import run_bass_kernel_spmd  # noqa: E402

N = 2048
FIN = 128
H = 8
NCORES = 8
R = N // NCORES          # 256 target rows per core
MT = N // 128            # 16 source m-tiles
FP32 = mybir.dt.float32
FP16 = mybir.dt.float16

# (name, fan_in, C) ; blocks: 0 = layers *1x (C=64), 1 = *2x (C=32)
LAYERS = [("11", 128, 64), ("12", 512, 64), ("13", 512, 64),
          ("21", 128, 32), ("22", 256, 32), ("23", 256, 32)]
# emission order interleaves the two independent blocks so one block's
# compute hides the other's AllGather transition
SCHED = [("11", 128, 64, None), ("21", 128, 32, None),
         ("12", 512, 64, "11"), ("22", 256, 32, "21"),
         ("13", 512, 64, "12"), ("23", 256, 32, "22")]

_NC_CACHE = {}
DEBUG = False
# pointwise variant per (m_tile % len): A=ACT prelu+exp; B=DVE lrelu + ACT
# exp-half + DVE square-mask; G2=DVE s02 + GP max + ACT exp + GP mask;
# GM=A but mask on GP.  hcopy engine: a=ACT, v=DVE.
VARIANTS = ["A", "GB", "B", "GB", "GB", "A", "GB", "B",
            "GB", "GB", "A", "GB", "GB", "B", "GB", "B"]
HCOPY = {"A": "a", "B": "a", "G2": "a", "GM": "a", "GB": "a"}


def _build(repeat=1, no_collective=False):
    nc = bacc.Bacc("TRN2", target_bir_lowering=False, debug=False,
                   num_devices=NCORES)

    # ---------------- DRAM I/O ----------------
    xT0_d = nc.dram_tensor("xT0", [FIN, N], FP16, kind="ExternalInput")
    xo0_d = nc.dram_tensor("xo0", [FIN, R], FP16, kind="ExternalInput")
    aT_d = nc.dram_tensor("aT", [N, R], FP16, kind="ExternalInput")
    es1_d = nc.dram_tensor("es1", [N], FP16, kind="ExternalInput")
    es2_d = nc.dram_tensor("es2", [N], FP16, kind="ExternalInput")
    wc_d, ws_d, b_d = {}, {}, {}
    for (nm, F, C) in LAYERS:
        HC = H * C
        # Wcat = [W.reshape(F, HC) | Wt]  (Wt = einsum(W, at))
        wc_d[nm] = nc.dram_tensor(f"Wc{nm}", [F, HC + H], FP16,
                                  kind="ExternalInput")
        ws_d[nm] = nc.dram_tensor(f"Ws{nm}", [F, H], FP16, kind="ExternalInput")
        b_d[nm] = nc.dram_tensor(f"b{nm}", [HC], FP32, kind="ExternalInput")
    pool_d = nc.dram_tensor("pool", [768], FP32, kind="ExternalOutput")

    dbg_d = {}
    if DEBUG:
        for li, (nm, F, C) in enumerate(LAYERS):
            OC = (H * C) // 128
            dbg_d[nm] = nc.dram_tensor(f"dbg{nm}", [128, OC * R], FP16,
                                       kind="ExternalOutput")

    # internal DRAM: es scratch per layer + allgather buffers per transition
    es_scr, ag_in, ag_out = {}, {}, {}
    for rep in range(repeat):
        for li, (nm, F, C) in enumerate(LAYERS):
            key = (rep, nm)
            es_scr[key] = nc.dram_tensor(f"esscr{rep}_{nm}", [N], FP16,
                                         kind="Internal")
            if li % 3 != 2:
                # carries h_own = xn @ W_next (+et cols) for the NEXT layer:
                # [own 256 rows, HC_next+H]; gathered -> h for all 2048 nodes
                nF, nC = LAYERS[[x[0] for x in LAYERS].index(nm) + 1][1:3]
                nW2 = H * nC + H
                ag_in[key] = nc.dram_tensor(f"agin{rep}_{nm}", [R, nW2], FP16,
                                            kind="Internal")
                ag_out[key] = nc.dram_tensor(
                    f"agout{rep}_{nm}", [NCORES * R, nW2], FP16,
                    kind="Internal", addr_space="Shared")

    with tile.TileContext(nc) as tc:
        with ExitStack() as ctx:
            pl = lambda **kw: ctx.enter_context(tc.tile_pool(**kw))  # noqa: E731
            constp = pl(name="const", bufs=1)
            wmp = pl(name="wm", bufs=2)
            wsmp = pl(name="wsm", bufs=2)
            xtb1p = pl(name="xtb1", bufs=1)
            xtb2p = pl(name="xtb2", bufs=1)
            hp = pl(name="hsb", bufs=6)
            esbp = pl(name="esb", bufs=1)
            essbp = pl(name="essb", bufs=2)
            etp = pl(name="etsb", bufs=4)
            sp = pl(name="s", bufs=4)
            up = pl(name="u", bufs=4)
            Pp = pl(name="P", bufs=4)
            pp = pl(name="p", bufs=6)
            xnp = pl(name="xn", bufs=2)
            dvp = pl(name="dv", bufs=8)
            dinvp = pl(name="dinv", bufs=2)
            dinvbp = pl(name="dinvb", bufs=2)
            poutp = pl(name="pout", bufs=1)
            php = pl(name="ph", bufs=2, space="PSUM")
            pep = pl(name="pe", bufs=1, space="PSUM")
            paggp = pl(name="pagg", bufs=4, space="PSUM")
            pDp = pl(name="pD", bufs=1, space="PSUM")

            # -------- prologue: resident inputs --------
            aT_sb = constp.tile([128, MT * R], FP16)          # [m | (mt, n)]
            aT_v = aT_d[:].rearrange("(t p) n -> p t n", p=128)
            _eng = [nc.gpsimd, nc.scalar, nc.gpsimd, nc.scalar]
            for j in range(4):
                _eng[j].dma_start(
                    aT_sb[:].rearrange("p (t n) -> p t n", t=MT)
                    [:, 4 * j:4 * j + 4, :],
                    aT_v[:, 4 * j:4 * j + 4, :])
            xT0_sb = constp.tile([128, N], FP16)
            nc.sync.dma_start(xT0_sb[:], xT0_d[:])
            xo0_sb = constp.tile([128, R], FP16)
            nc.sync.dma_start(xo0_sb[:], xo0_d[:])
            ones_sb = constp.tile([128, 1], FP16)
            nc.gpsimd.memset(ones_sb[:], 1.0)

            for rep in range(repeat):
                xn_prev = [None, None]   # per block: own transposed activation
                for li, (nm, F, C, prev) in enumerate(SCHED):
                    HC = H * C
                    FC = F // 128        # input chunks (of fan-in)
                    OC = HC // 128       # output chunks (of hc rows)
                    W2 = HC + H          # wcat width
                    blk = 0 if nm[0] == "1" else 1
                    lyr = int(nm[1]) - 1
                    fuse_et = (HC + H) <= 512   # block2: et inside h-matmul

                    # -------- weights (wc only for layer-0 h matmuls) -----
                    if lyr == 0:
                        wc_sb = wmp.tile([128, FC * W2], FP16, tag="wm")
                        nc.gpsimd.dma_start(
                            wc_sb[:].rearrange("p (c d) -> p c d", c=FC),
                            wc_d[nm][:].rearrange("(c p) d -> p c d", p=128))
                    ws_sb = wsmp.tile([128, FC * H], FP16, tag="ws")
                    nc.gpsimd.dma_start(
                        ws_sb[:].rearrange("p (c d) -> p c d", c=FC),
                        ws_d[nm][:].rearrange("(c p) d -> p c d", p=128))
                    b_sb = wsmp.tile([128, OC], FP32, tag="b")
                    nc.gpsimd.dma_start(
                        b_sb[:], b_d[nm][:].rearrange("(c p) -> p c", p=128))

                    # -------- h source --------
                    # lyr==0: compute h from resident xT0 per m-tile (below).
                    # lyr>0: h (+et cols) was computed by the previous layer's
                    # producer and AllGather'd as [2048 nodes, W2]; load it
                    # directly per 2-m-tile chunk -- no recompute, no copies.
                    if lyr == 0:
                        xT_sb = xT0_sb
                        xo_ap = xo0_sb
                    else:
                        xo_ap = xn_prev[blk]
                        gsrc = ag_out[(rep, prev)]
                        hg = []
                        for g in range(MT // 2):
                            h2 = hp.tile([128, 2 * W2], FP16, tag="h")
                            nc.sync.dma_start(
                                h2[:].rearrange("p (t d) -> p t d", t=2),
                                gsrc[:].rearrange("(t p) d -> p t d", p=128)
                                [:, 2 * g:2 * g + 2, :])
                            hg.append(h2)

                    # -------- es chain --------
                    if lyr == 0:
                        es_src = es1_d if blk == 0 else es2_d
                    else:
                        es_src = es_scr[(rep, nm)]
                        pes = pep.tile([8, R], FP32, tag="pe")
                        for fc in range(FC):
                            nc.tensor.matmul(
                                pes[:], ws_sb[:, fc * H:(fc + 1) * H],
                                xo_ap[:, fc * R:(fc + 1) * R],
                                start=(fc == 0), stop=(fc == FC - 1))
                        es_sb = essbp.tile([8, R], FP16, tag="es")
                        nc.scalar.copy(es_sb[:], pes[:])
                        nc.gpsimd.dma_start(
                            es_src[:].rearrange("(h n) -> h n", h=8), es_sb[:])
                    esb = esbp.tile([128, N], FP16, tag="esb")
                    for j in range(2):
                        nc.sync.dma_start(
                            esb[:, j * 1024:(j + 1) * 1024],
                            es_src[j * 1024:(j + 1) * 1024][None, :]
                            .to_broadcast((128, 1024)))

                    # -------- aggregation psum (live across m loop) --------
                    agg_q = []
                    n_agg = 4 if C == 64 else 2
                    for _qi in range(n_agg):
                        agg_t = paggp.tile([128, 512], FP32, tag="agg")
                        agg_q.append(agg_t)
                    pD = pDp.tile([128, 512], FP32, tag="pD")

                    for i in range(MT):
                        # h (+ et fused for block2) for m-tile i
                        var = VARIANTS[i % len(VARIANTS)]
                        et_t = etp.tile([128, H], FP32, tag="et")
                        if lyr == 0:
                            ph = php.tile([128, W2 if fuse_et else HC], FP32,
                                          tag="ph")
                            rw = W2 if fuse_et else HC
                            for fc in range(FC):
                                lhs = xT_sb[:, fc * N + i * 128:
                                            fc * N + (i + 1) * 128]
                                nc.tensor.matmul(
                                    ph[:], lhs,
                                    wc_sb[:, fc * W2: fc * W2 + rw],
                                    start=(fc == 0), stop=(fc == FC - 1))
                            if not fuse_et:
                                pet = pep.tile([128, H], FP32, tag="pe")
                                for fc in range(FC):
                                    lhs = xT_sb[:, fc * N + i * 128:
                                                fc * N + (i + 1) * 128]
                                    nc.tensor.matmul(
                                        pet[:], lhs,
                                        wc_sb[:, fc * W2 + HC: (fc + 1) * W2],
                                        start=(fc == 0), stop=(fc == FC - 1))
                            h_sb = hp.tile([128, HC], FP16, tag="h")
                            nc.scalar.copy(h_sb[:], ph[:, 0:HC])
                            nc.scalar.copy(et_t[:], ph[:, HC:W2] if fuse_et
                                           else pet[:])
                            h_ap = h_sb[:]
                        else:
                            co = (i % 2) * W2
                            h_ap = hg[i // 2][:, co:co + HC]
                            nc.scalar.copy(
                                et_t[:], hg[i // 2][:, co + HC:co + W2])
                        et_ap = et_t

                        # pointwise: p = aT * exp(lrelu_0.2(es + et))
                        # variant A (ACT-heavy): Prelu + Exp on ACT, mask DVE
                        # variant B (DVE-heavy): lrelu = max(s, .2s) on DVE,
                        #   exp(u/2) on ACT, then p = (A2*aT)*A2 on DVE
                        if var not in ("A", "GM"):
                            s_t = sp.tile([128, N], FP16, tag="s")
                            for h in range(H):
                                nc.vector.tensor_scalar(
                                    s_t[:, h * R:(h + 1) * R],
                                    esb[:, h * R:(h + 1) * R],
                                    et_ap[:, h: h + 1], None,
                                    mybir.AluOpType.add)
                        aT_ap = (aT_sb[:, i * R:(i + 1) * R][:, None, :]
                                 .to_broadcast((128, H, R)))
                        p_t = pp.tile([128, N], FP16, tag="p")
                        if var in ("A", "GM"):
                            # es+et add folded into the Prelu bias (per head)
                            # -- no DVE s-adds for these tiles
                            u_t = up.tile([128, N], FP16, tag="u")
                            for h in range(H):
                                nc.scalar.activation(
                                    u_t[:, h * R:(h + 1) * R],
                                    esb[:, h * R:(h + 1) * R],
                                    mybir.ActivationFunctionType.Prelu,
                                    bias=et_ap[:, h:h + 1], alpha=0.2)
                            P_t = Pp.tile([128, N], FP16, tag="P")
                            nc.scalar.activation(
                                P_t[:], u_t[:],
                                mybir.ActivationFunctionType.Exp)
                            eng = nc.vector if var == "A" else nc.gpsimd
                            eng.tensor_tensor(
                                p_t[:].rearrange("p (h n) -> p h n", h=H),
                                P_t[:].rearrange("p (h n) -> p h n", h=H),
                                aT_ap, mybir.AluOpType.mult)
                        elif var == "GB":
                            # DVE lrelu, ACT exp, GP mask
                            s2_t = up.tile([128, N], FP16, tag="u")
                            nc.vector.tensor_scalar(
                                s2_t[:], s_t[:], 0.2, None,
                                mybir.AluOpType.mult)
                            u2_t = up.tile([128, N], FP16, tag="u")
                            nc.vector.tensor_tensor(
                                u2_t[:], s_t[:], s2_t[:],
                                mybir.AluOpType.max)
                            P_t = Pp.tile([128, N], FP16, tag="P")
                            nc.scalar.activation(
                                P_t[:], u2_t[:],
                                mybir.ActivationFunctionType.Exp)
                            nc.gpsimd.tensor_tensor(
                                p_t[:].rearrange("p (h n) -> p h n", h=H),
                                P_t[:].rearrange("p (h n) -> p h n", h=H),
                                aT_ap, mybir.AluOpType.mult)
                        elif var == "B":
                            # lrelu on DVE (max(s, .2s)), exp on ACT, mask DVE
                            s2_t = up.tile([128, N], FP16, tag="u")
                            nc.vector.tensor_scalar(
                                s2_t[:], s_t[:], 0.2, None,
                                mybir.AluOpType.mult)
                            u2_t = up.tile([128, N], FP16, tag="u")
                            nc.vector.tensor_tensor(
                                u2_t[:], s_t[:], s2_t[:],
                                mybir.AluOpType.max)
                            P_t = Pp.tile([128, N], FP16, tag="P")
                            nc.scalar.activation(
                                P_t[:], u2_t[:],
                                mybir.ActivationFunctionType.Exp)
                            nc.vector.tensor_tensor(
                                p_t[:].rearrange("p (h n) -> p h n", h=H),
                                P_t[:].rearrange("p (h n) -> p h n", h=H),
                                aT_ap, mybir.AluOpType.mult)
                        else:  # G2: DVE s02, GP max, ACT exp, GP mask
                            s2_t = up.tile([128, N], FP16, tag="u")
                            nc.vector.tensor_scalar(
                                s2_t[:], s_t[:], 0.2, None,
                                mybir.AluOpType.mult)
                            nc.gpsimd.tensor_tensor(
                                s_t[:], s_t[:], s2_t[:],
                                mybir.AluOpType.max)
                            P_t = Pp.tile([128, N], FP16, tag="P")
                            nc.scalar.activation(
                                P_t[:], s_t[:],
                                mybir.ActivationFunctionType.Exp)
                            nc.gpsimd.tensor_tensor(
                                p_t[:].rearrange("p (h n) -> p h n", h=H),
                                P_t[:].rearrange("p (h n) -> p h n", h=H),
                                aT_ap, mybir.AluOpType.mult)

                        # aggregation: 2 heads per matmul ([128, 512] rhs).
                        # One accumulation group per (partition-range, bank).
                        for j in range(4):
                            lhsT = h_ap[:, j * 2 * C:(j + 1) * 2 * C]
                            rhs = p_t[:, j * 512:(j + 1) * 512]
                            if C == 64:
                                nc.tensor.matmul(
                                    agg_q[j][:, :], lhsT, rhs,
                                    start=(i == 0), stop=(i == MT - 1),
                                    tile_position=(0, 0))
                            else:
                                pb = (j % 2) * 64
                                nc.tensor.matmul(
                                    agg_q[j // 2][pb:pb + 64, :], lhsT, rhs,
                                    start=(i == 0), stop=(i == MT - 1),
                                    tile_position=(0, pb),
                                    skip_group_check=(pb > 0))
                        for j in range(4):
                            nc.tensor.matmul(
                                pD[32 * j:32 * j + 1, :],
                                ones_sb[:],
                                p_t[:, j * 512:(j + 1) * 512],
                                start=(i == 0), stop=(i == MT - 1),
                                tile_position=(0, 32 * j),
                                skip_group_check=(j > 0))

                    # -------- finalize: alpha-normalize + bias + relu ------
                    # pipelined per head-pair: recip chunk (DVE) -> broadcast
                    # chunk (GP, from partition 0) -> normalize+relu, so the
                    # ag_in payload is ready ~8us earlier than monolithic
                    # stages and the AllGather starts sooner
                    dinv = dinvp.tile([1, N], FP32, tag="dinv")
                    dinvb = dinvbp.tile([128, N], FP32, tag="dinvb")
                    xn = xnp.tile([128, OC * R], FP16, tag=f"xn{blk}")
                    hpc = 128 // C  # heads per 128-row chunk
                    for j in range(4):
                        sl = slice(j * 512, (j + 1) * 512)
                        nc.vector.reciprocal(dinv[0:1, sl],
                                             pD[32 * j:32 * j + 1, :])
                        nc.gpsimd.partition_broadcast(dinvb[:, sl],
                                                      dinv[0:1, sl])
                        for h in (2 * j, 2 * j + 1):
                            t, k = divmod(h, hpc)
                            pb = k * C
                            fo = (k % 2) * 256
                            # b1: chunk t = pair tile t; b2: tile t, see map
                            src = agg_q[t][pb:pb + C, fo:fo + R]
                            dv = dvp.tile([128, R], FP32, tag="dv")
                            nc.vector.tensor_tensor(
                                dv[pb:pb + C, :], src,
                                dinvb[pb:pb + C, h * R:(h + 1) * R],
                                mybir.AluOpType.mult)
                            nc.scalar.activation(
                                xn[pb:pb + C, t * R:(t + 1) * R],
                                dv[pb:pb + C, :],
                                mybir.ActivationFunctionType.Relu,
                                bias=b_sb[pb:pb + C, t:t + 1])

                    if DEBUG:
                        nc.sync.dma_start(dbg_d[nm][:, 0:OC * R], xn[:])
                    if lyr == 2:
                        # global pool: partial sum over own 256 rows
                        po = poutp.tile([128, OC], FP32, tag=f"po{blk}")
                        for t in range(OC):
                            nc.vector.tensor_reduce(
                                po[:, t:t + 1], xn[:, t * R:(t + 1) * R],
                                axis=mybir.AxisListType.X,
                                op=mybir.AluOpType.add)
                        off = 0 if blk == 0 else 512
                        nc.sync.dma_start(
                            pool_d[off:off + HC].rearrange("(c p) -> p c",
                                                           p=128),
                            po[:])
                    else:
                        xn_prev[blk] = xn
                        # producer: next layer's h_own = xn @ Wc_next for the
                        # own 256 rows only; the AllGather then distributes
                        # finished h (+et cols), so no core recomputes h for
                        # all 2048 nodes.  Output dims (HC, W2) match this
                        # layer's (C constant within a block).
                        nmx = nm[0] + str(int(nm[1]) + 1)
                        wcn = wmp.tile([128, OC * W2], FP16, tag="wm")
                        nc.gpsimd.dma_start(
                            wcn[:].rearrange("p (c d) -> p c d", c=OC),
                            wc_d[nmx][:].rearrange("(c p) d -> p c d", p=128))
                        for mh in range(2):
                            phh = php.tile([128, HC], FP32, tag="ph")
                            pee = pep.tile([128, H], FP32, tag="pe")
                            for fc in range(OC):
                                lhs = xn[:, fc * R + mh * 128:
                                         fc * R + mh * 128 + 128]
                                nc.tensor.matmul(
                                    phh[:], lhs,
                                    wcn[:, fc * W2:fc * W2 + HC],
                                    start=(fc == 0), stop=(fc == OC - 1))
                                nc.tensor.matmul(
                                    pee[:], lhs,
                                    wcn[:, fc * W2 + HC:(fc + 1) * W2],
                                    start=(fc == 0), stop=(fc == OC - 1))
                            h8 = hp.tile([128, W2], FP16, tag="h")
                            nc.scalar.copy(h8[:, 0:HC], phh[:])
                            nc.scalar.copy(h8[:, HC:W2], pee[:])
                            nc.sync.dma_start(
                                ag_in[(rep, nm)][mh * 128:(mh + 1) * 128, :],
                                h8[:])
                        if no_collective:
                            for r in range(NCORES):
                                nc.sync.dma_start(
                                    ag_out[(rep, nm)][r * HC:(r + 1) * HC, :],
                                    ag_in[(rep, nm)][:])
                        else:
                            nc.gpsimd.collective_compute(
                                "AllGather", mybir.AluOpType.bypass,
                                replica_groups=[list(range(NCORES))],
                                ins=[ag_in[(rep, nm)][:].opt()],
                                outs=[ag_out[(rep, nm)][:].opt()])

    nc.compile()
    return nc


def _get_nc():
    if "nc" not in _NC_CACHE:
        _NC_CACHE["nc"] = _build()
    return _NC_CACHE["nc"]


def _prep_inputs(inputs):
    f16 = np.float16
    x = np.asarray(inputs["x"], np.float32)
    a = np.asarray(inputs["a"], np.float32)
    base = {}
    base["xT0"] = np.ascontiguousarray(x.T).astype(f16)
    for (nm, F, C) in LAYERS:
        W = np.asarray(inputs["W" + nm], np.float32)   # [F, H, C]
        at = np.asarray(inputs["at" + nm], np.float32)  # [H, C]
        as_ = np.asarray(inputs["as" + nm], np.float32)
        wt = np.einsum("fhc,hc->fh", W, at)
        wcat = np.concatenate([W.reshape(F, H * C), wt], axis=1)
        base["Wc" + nm] = np.ascontiguousarray(wcat).astype(f16)
        base["Ws" + nm] = np.ascontiguousarray(
            np.einsum("fhc,hc->fh", W, as_)).astype(f16)
        base["b" + nm] = np.asarray(inputs["b" + nm], np.float32)
    maps = []
    xb = x.astype(np.float16).astype(np.float32)  # match device fp16
    for c in range(NCORES):
        m = dict(base)
        m["aT"] = np.ascontiguousarray(a[c * R:(c + 1) * R, :].T).astype(f16)
        m["xo0"] = np.ascontiguousarray(x[c * R:(c + 1) * R, :].T).astype(f16)
        xo = xb[c * R:(c + 1) * R, :]
        for blk, nm in ((0, "11"), (1, "21")):
            W = np.asarray(inputs["W" + nm], np.float32)
            as_ = np.asarray(inputs["as" + nm], np.float32)
            ws = np.einsum("fhc,hc->fh", W, as_)
            ws = ws.astype(np.float16).astype(np.float32)
            es = xo @ ws                       # [R, H]
            m["es1" if blk == 0 else "es2"] = np.ascontiguousarray(
                es.T.reshape(-1)).astype(np.float16)
        maps.append(m)
    return maps


def kernel(**inputs):
    nc = _get_nc()
    maps = _prep_inputs(inputs)
    res = run_bass_kernel_spmd(nc, maps, core_ids=list(range(NCORES)))
    out = np.zeros(768, np.float64)
    for c in range(NCORES):
        out += res.results[c]["pool"].astype(np.float64)
    return out.astype(np.float32)


if __name__ == "__main__":
    rng = np.random.default_rng(0)
    ins = {"x": rng.standard_normal((N, FIN)).astype(np.float32),
           "a": (rng.random((N, N)) < 0.01).astype(np.float32)}
    for (nm, F, C) in LAYERS:
        ins["W" + nm] = (rng.standard_normal((F, H, C)) / np.sqrt(F)).astype(np.float32)
        ins["as" + nm] = (rng.standard_normal((H, C)) * 0.1).astype(np.float32)
        ins["at" + nm] = (rng.standard_normal((H, C)) * 0.1).astype(np.float32)
        ins["b" + nm] = np.zeros(H * C, np.float32)
    out = kernel(**ins)
    print("kernel out[:8] =", out[:8])

